# revision 48
# baseline (speedup 1.0000x reference)
"""Trainium2 Bass kernel for prefix-KV multi-head attention (v3).

Reference computation (per batch):
    qkv = x @ w_qkv -> q,k,v heads; k/v get a 16-token prefix (pk, pv)
    attn = softmax(q @ k^T * D^-0.5); out = (attn @ v) @ w_proj + b_proj

Sharding: data-parallel over B across 8 NeuronCores (2 batches per core).

Design (v1 700us -> v2 617us -> v3 576us, all HW-measured):
  - weights resident in SBUF (bf16), loaded in 512-col chunks on the
    gpsimd sw-DGE queue (128-col chunks made 256B write packets and left
    the queue PACKET-RATE-bound ~60us; 1KB packets finish in ~25us),
    demand-ordered: pair-0/1 q/k, v block 0, pv, v block 1, rest, wproj
  - x rides the separate sync HW queue (4KB packets) concurrently,
    fp32 -> ACT cast -> bf16 PE transposes -> xT
  - preamble split by token half: the qk GEMM for half jh only reads
    x tiles 4jh..4jh+3, so qk/v GEMMs start after FOUR tiles instead of
    eight (PE chews while tiles 4-7 stream in)
  - a 32-MM warm-up burst at t=0 releases the HAM clock gate (PE is
    throttled to 1.2 GHz until ~3.4us of sustained activity)
  - qT/kT hold FOUR head pairs (slot p%4); pair p+2 is produced by
    pipelined fillers during pair p (urgent queue, 4 units/pair at mt
    slots 1/5); v block 1 + leftovers drain as lazy fillers (mt 3/7)
  - PACKED prefix: the 16 prefix keys of 4 heads land on 32-row stripes
    of ONE [128,1024] psum (stationary kPre is 32 wide with zero pad;
    explicit tile_position=(base,32*hg), row-half-major MM order so only
    verified-safe masked||masked overlap occurs) -> ONE exp serves 4
    heads instead of 4; v_ext m-tile 0 holds pv_h on partitions
    32*(h%4)..+16, zeros elsewhere, so other heads' e values in the
    shared e_pre contribute nothing
  - per-head attention over m-tiles 1..8 (tokens only): psS 2x2 banks
    double-buffered scores + psAV 2 banks av accumulator + psG 2x1
    gemm scratch = 8 banks exactly
  - ones-columns in v_ext give the softmax denominator for free
    (output ROWS of a matmul are free; cost = moving columns)
  - softmax 1/denominator via exp(-ln(d)) on ACT; a DVE copy of the
    numerator releases the av psum ~1.1us early
  - proj passes of the previous batch carry into the next preamble

Measured dead ends (don't retry without new evidence):
  - XBAR DMA transpose for x^T: 208B packets, 855us total (vs 576)
  - fold next batch's tiles/qk into this batch's tail pairs + split
    proj into ct-halves: 587-599us over four runs -- the attention
    phase is PE-saturated, so folded work stretches spans ~1:1 and the
    inter-batch "gap" was already dense PE work
  - x via casting gpsimd DMA (no fp32 staging): x cadence 2.3us/tile
    (vs 1.4 on sync queue) and it serializes behind weight chunks
  - DVE cast for xbf: DVE FIFO serializes with xT/qk copies, +7us
  - issuing av-mt0 after scores-mt1 (to hide the psAV-release wait):
    +9us regression
  - writing v_ext (a STATIONARY operand) inside the consuming head's
    slots: rel-err 0.56/NaN -- LDWEIGHTS pull-ahead (64-deep window)
    loads stale data, ignoring semaphore order; stationary operands
    must be written well outside the consumer's instruction window
  - v2's rejected list still stands: fp8 (5.7% err vs 2% tol), Pool
    normalize (742us), full score-pair row-packing with full-array av
    behind it (corrupts unless sync-guarded; guarded 641-759us)

Roofline notes (per core): PE ~540us active of 576 (94%); ACT exp
floor ~8.8us/head (1.18M elems / 128 lanes / 1.2GHz, dtype-blind) +
~2.3us/head normalize; per-head span ~12.4us vs ACT floor ~11.5 --
both engines are within ~10% of saturation, so further gains need the
score matmuls packed 2x (K=64) AND a cheaper normalize together.

Beware when benchmarking: after ~1h of sustained runs the chip enters
the P0 power state (PE 2.4 -> ~2.0 GHz; every engine ~19% slower, HAM
still shows K=8/8) and identical code measures ~688us instead of ~577.

This file is self-contained: it monkeypatches two workarounds for the
walrus build in this container (1-sync-wait-per-instruction cap).
"""

import json
import os
import sys
from collections import deque

for _p in ("/opt/trn_rl_repo", os.path.expanduser("~/.axon_site/_ro/trn_rl_repo")):
    if os.path.isdir(_p) and _p not in sys.path:
        sys.path.insert(0, _p)

import numpy as np

import concourse.bass as bass
import concourse.tile as tile
from concourse import mybir
from concourse.bass_utils import run_bass_kernel_spmd
from concourse.vector_clock import ScopedClock
from concourse.masks import make_identity

F32 = mybir.dt.float32
BF16 = mybir.dt.bfloat16
AF = mybir.ActivationFunctionType

# ---------------------------------------------------------------------------
# Workaround: this container's walrus supports at most ONE sync wait per
# instruction.  (a) split the TileContext-exit drain's waits onto single-wait
# NOPs; (b) at BIR-JSON serialization time, hoist extra waits from any
# instruction onto same-engine NOPs placed immediately before it.
# ---------------------------------------------------------------------------

def _patched_drain_and_barrier(self, tick_clock, wait_clock):
    drain_inst = self.nc.sync.drain()
    wait_clock.add_sem_waits(
        drain_inst.ins, ScopedClock({None: tick_clock.global_clock})
    )
    si = drain_inst.ins.sync_info
    waits = list(si.on_wait) if si is not None and si.on_wait else []
    if len(waits) > 1:
        si.on_wait = waits[:1]
        for w in waits[1:]:
            nop = self.nc.sync.nop(hint="drain_wait_split", nofuse=True)
            nsi = nop.ins.sync_info
            if nsi is None:
                nop.ins.sync_info = mybir.SyncInfo(on_wait=[w], on_update=[])
            else:
                nsi.on_wait = list(nsi.on_wait or []) + [w]
    self.nc.all_engine_barrier()
    assert self.sems is not None
    popped = self.nc._tile_sem_poison_stack.pop()
    assert popped is self._sem_poison
    self.nc.clear_and_free_semaphores(list(self.sems.allocated().values()))
    self.nc.all_engine_barrier()


tile.TileContext._drain_and_barrier = _patched_drain_and_barrier


def _split_multi_waits(bir):
    for fn in bir["functions"]:
        for bb in fn["blocks"]:
            new_insts = []
            for inst in bb["instructions"]:
                si = inst.get("sync_info")
                ow = (si or {}).get("on_wait") or []
                if len(ow) > 1:
                    for i, w in enumerate(ow[:-1]):
                        new_insts.append({
                            "debug": inst.get("debug", 0),
                            "engine": inst["engine"],
                            "ins": [], "outs": [],
                            "name": f"{inst['name']}.wsplit{i}",
                            "opcode": "NoOp",
                            "sync_info": {"on_wait": [w], "on_update": []},
                        })
                    si["on_wait"] = [ow[-1]]
                new_insts.append(inst)
            bb["instructions"] = new_insts
    return bir


_orig_to_json_bytes = bass.Bass.to_json_bytes


def _patched_to_json_bytes(self):
    d = json.loads(_orig_to_json_bytes(self))
    _split_multi_waits(d)
    return json.dumps(d).encode()


bass.Bass.to_json_bytes = _patched_to_json_bytes

# ---------------------------------------------------------------------------
# Problem constants (hardcoded per the task contract)
# ---------------------------------------------------------------------------

B, N, C, H, P = 16, 1024, 1024, 16, 16
D = C // H                      # 64
SCALE = float(D) ** -0.5        # 0.125
N_CORES = 8
B_PC = B // N_CORES             # 2 batches per core
NT = N // 128                   # 8 token tiles
CT = C // 128                   # 8 feature tiles
MT = NT + 1                     # 9 m-tiles: tile 0 = prefix (16 valid rows)
HPAIRS = H // 2                 # 8 head pairs
FOLD_NEXT = True


def build_nc(repeat: int = 1) -> bass.Bass:
    nc = bass.Bass()

    x_d = nc.declare_dram_parameter("x", [B_PC, N, C], F32, isOutput=False)
    pk_d = nc.declare_dram_parameter("pk", [B_PC, P, C], F32, isOutput=False)
    pv_d = nc.declare_dram_parameter("pv", [B_PC, P, C], F32, isOutput=False)
    wqkv_d = nc.declare_dram_parameter("w_qkv", [C, 3 * C], F32, isOutput=False)
    wproj_d = nc.declare_dram_parameter("w_proj", [C, C], F32, isOutput=False)
    bias_d = nc.declare_dram_parameter("b_proj", [C], F32, isOutput=False)
    # output is stored TRANSPOSED per batch: [C, N]; host transposes back
    outT_d = nc.declare_dram_parameter("outT", [B_PC, C, N], F32, isOutput=True)

    with tile.TileContext(nc) as tc:
        with tc.tile_pool(name="cons", bufs=1) as cons, \
             tc.tile_pool(name="eP", bufs=4) as e_pool, \
             tc.tile_pool(name="ePre", bufs=2) as epre_pool, \
             tc.tile_pool(name="stg", bufs=1) as stg, \
             tc.tile_pool(name="rbp", bufs=1) as rb_pool, \
             tc.tile_pool(name="xload", bufs=3) as xload, \
             tc.tile_pool(name="xbf", bufs=2) as xbfp, \
             tc.tile_pool(name="osb", bufs=2) as osb, \
             tc.tile_pool(name="psS", bufs=2, space="PSUM") as psS, \
             tc.tile_pool(name="psAV", bufs=1, space="PSUM") as psAV, \
             tc.tile_pool(name="psG", bufs=2, space="PSUM") as psG:

            # ---------------- one-time setup ----------------
            ident_bf = cons.tile([128, 128], BF16, tag="idb")
            make_identity(nc, ident_bf[:])
            # PE warm-up burst: ~3.5us of throwaway matmuls releases the
            # HAM clock-gate (K=4/8 -> 8/8) before the real work arrives,
            # so the preamble transposes/GEMMs run at 2.4 GHz not 1.2.
            warm_ps = psG.tile([128, 128], F32, tag="g", name="warmup")
            for _w in range(32):
                nc.tensor.matmul(
                    warm_ps[:], ident_bf[:], ident_bf[:],
                    start=(_w == 0), stop=(_w == 31),
                )
            # bias in per-partition layout: bias_col[p, cf] = b_proj[cf*128+p]
            bias_col = cons.tile([128, CT], F32, tag="bias")
            nc.sync.dma_start(
                out=bias_col[:],
                in_=bias_d[:].rearrange("(a b) -> b a", b=128),
            )
            # prefix-k staging (bf16 via casting gpsimd DMA)
            pkl = cons.tile([P, C], BF16, tag="pkl")

            # persistent activations (reused across batches; Tile tracks
            # read/write hazards on AP ranges).  qT/kT hold THREE head
            # pairs (slot p%3): pair p+2 is produced by pipelined fillers
            # while pair p's attention reads its slot; the extra slot lets
            # the packed-prefix exp (4 heads = 2 pairs per ACTIVATE) see
            # both of its pairs' q at group start.
            xT = cons.tile([128, CT, N], BF16, tag="xT")
            kT = cons.tile([128, 4, N], BF16, tag="kT")
            qT = cons.tile([128, 4, N], BF16, tag="qT")
            # prefix keys, all pairs: cols 0:16 = pk^T, 16:32 zero so the
            # packed 32-row score stripes come out 0 on rows 16:32 ->
            # exp = 1, harmless because the matching v_ext rows are zero
            kPre = cons.tile([128, HPAIRS, 32], BF16, tag="kPre")
            nc.vector.memset(kPre[:, :, P:32], 0.0)
            oT = cons.tile([128, CT, N], BF16, tag="oT")
            # v_ext[m, mt, h, 0:64] = v values; [.., 64:128] = ones columns
            # (denominator trick). m-tile 0 = prefix, PACKED: head h's 16
            # pv rows live at partitions 32*(h%4)..+16 (matching its stripe
            # in the packed prefix-score psum); all other rows stay ZERO so
            # the other heads' e values in the shared e_pre tile contribute
            # nothing to this head's av or denominator.
            v_ext = cons.tile([128, MT, H, 128], BF16, tag="vx")
            nc.vector.memset(v_ext[:, :, :, 64:128], 1.0)
            nc.vector.memset(v_ext[:, 0, :, :], 0.0)
            for a in range(4):
                nc.vector.memset(
                    v_ext[32 * a:32 * a + P, 0, a::4, 64:128], 1.0
                )

            # weights, bf16, resident for the whole kernel, on the gpsimd
            # sw-DGE queue (the only one that casts).  512-col chunks keep
            # the write packets at 1KB (128-col chunks made 256B packets and
            # left the queue packet-rate-bound for ~60us).  x rides the
            # separate sync HW queue concurrently.
            wq_sb = cons.tile([128, CT, C], BF16, tag="wq")
            wk_sb = cons.tile([128, CT, C], BF16, tag="wk")
            wv_sb = cons.tile([128, CT, C], BF16, tag="wv")
            wp_sb = cons.tile([128, CT, C], BF16, tag="wp")

            def _wload(dst, base, lo, hi):
                nc.gpsimd.dma_start(
                    out=dst[:, :, lo:hi],
                    in_=wqkv_d[:, base + lo:base + hi].rearrange(
                        "(ct p) f -> p ct f", p=128),
                )

            def _pv_load(b):
                pvr = pv_d[b].rearrange("t (h d) -> t h d", d=64)
                for a in range(4):
                    nc.gpsimd.dma_start(
                        out=v_ext[32 * a:32 * a + P, 0, a::4, 0:64],
                        in_=pvr[:, a::4, :],
                    )

            nc.gpsimd.dma_start(out=pkl[:], in_=pk_d[0])
            _wload(wk_sb, C, 0, 128)                  # k pair 0
            _wload(wq_sb, 0, 0, 128)                  # q pair 0
            _wload(wv_sb, 2 * C, 0, 512)              # v block 0 (needed
            # by the preamble v-units ~20us in; pair-1 q/k isn't consumed
            # until the qk1-jh0 units ~23us, so v goes first)
            _wload(wq_sb, 0, 128, 256)                # q pair 1
            _wload(wk_sb, C, 128, 256)                # k pair 1
            _pv_load(0)                               # prefix v, batch 0
            _wload(wv_sb, 2 * C, 512, 1024)           # v block 1
            _wload(wk_sb, C, 256, 640)
            _wload(wq_sb, 0, 256, 640)
            _wload(wk_sb, C, 640, 1024)
            _wload(wq_sb, 0, 640, 1024)
            nc.gpsimd.dma_start(
                out=wp_sb[:],
                in_=wproj_d[:].rearrange("(ct p) f -> p ct f", p=128),
            )

            # ---------------- per-batch work units ----------------

            def qk_units(b, p):
                """4 closures: q and k GEMMs for head pair p, split in two
                512-column halves each. Each accumulates 8 c-tiles into a
                [128,512] psum and copies (cast bf16) into qT/kT."""
                us = []
                for which in ("k", "q"):
                    for jh in range(2):
                        def u(which=which, p=p, jh=jh, b=b):
                            w_sb = wk_sb if which == "k" else wq_sb
                            ps = psG.tile([128, 512], F32, tag="g",
                                          name=f"g{which}_{b}_{p}_{jh}")
                            for ct in range(CT):
                                nc.tensor.matmul(
                                    ps[:],
                                    w_sb[:, ct, p * 128:(p + 1) * 128],
                                    xT[:, ct, jh * 512:(jh + 1) * 512],
                                    start=(ct == 0), stop=(ct == CT - 1),
                                )
                            if which == "k":
                                nc.vector.tensor_copy(
                                    kT[:, p % 4, jh * 512:(jh + 1) * 512],
                                    ps[:],
                                )
                            else:
                                nc.vector.tensor_copy(
                                    qT[:, p % 4, jh * 512:(jh + 1) * 512],
                                    ps[:],
                                )
                        us.append(u)
                return us

            def v_units(b, bk):
                """8 closures: v GEMM for pair block bk (4 pairs = 512 v
                columns), one per token tile. x^T tile is stationary, w_v
                columns are moving -> v lands in NATURAL [token, feature]
                layout, no transpose needed."""
                us = []
                for nt in range(NT):
                    def u(nt=nt, bk=bk, b=b):
                        ps = psG.tile([128, 512], F32, tag="g",
                                      name=f"gv_{b}_{bk}_{nt}")
                        for ct in range(CT):
                            nc.tensor.matmul(
                                ps[:],
                                xT[:, ct, nt * 128:(nt + 1) * 128],
                                wv_sb[:, ct, bk * 512:(bk + 1) * 512],
                                start=(ct == 0), stop=(ct == CT - 1),
                            )
                        nc.vector.tensor_copy(
                            v_ext[:, nt + 1, 8 * bk:8 * (bk + 1), 0:64],
                            ps[:].rearrange("p (h d) -> p h d", d=64),
                        )
                    us.append(u)
                return us

            def proj_units(b):
                """8 closures: one projection f-tile pass each; emitted
                interleaved into the NEXT batch's preamble."""
                us = []
                for cf in range(CT):
                    def u(cf=cf, b=b):
                        ps = psS.tile([128, N], F32, tag="s",
                                      name=f"pp_{b}_{cf}")
                        for ct in range(CT):
                            for j in (0, 512):
                                nc.tensor.matmul(
                                    ps[:, j:j + 512],
                                    wp_sb[:, ct, cf * 128:(cf + 1) * 128],
                                    oT[:, ct, j:j + 512],
                                    start=(ct == 0), stop=(ct == CT - 1),
                                )
                        o_sb = osb.tile([128, N], F32, tag="o",
                                        name=f"osb_{b}_{cf}")
                        nc.vector.tensor_scalar_add(
                            o_sb[:], ps[:], bias_col[:, cf:cf + 1]
                        )
                        nc.sync.dma_start(
                            out=outT_d[b, cf * 128:(cf + 1) * 128, :],
                            in_=o_sb[:],
                        )
                    us.append(u)
                return us

            def tile_unit(b, nt):
                """x tile -> bf16 -> x^T (sync-HW-queue DMA, ACT cast, PE
                transposes).  Returned as a closure so the NEXT batch's
                tiles can run as lazy filler in THIS batch's tail pairs.
                (XBAR DMA transpose was measured: 208B packets, 855us total
                -- the PE path is far faster for 128x128 tiles.)"""
                def u():
                    xl = xload.tile([128, C], F32, tag="xl",
                                    name=f"xl_{b}_{nt}")
                    nc.sync.dma_start(
                        out=xl[:], in_=x_d[b, nt * 128:(nt + 1) * 128, :]
                    )
                    xbf = xbfp.tile([128, C], BF16, tag="xbf",
                                    name=f"xbf_{b}_{nt}")
                    nc.scalar.activation(xbf[:], xl[:], AF.Copy)
                    ps_t = psG.tile([128, CT, 128], BF16, tag="g",
                                    name=f"pst_{b}_{nt}")
                    for ct in range(CT):
                        nc.tensor.transpose(
                            ps_t[:, ct, :],
                            xbf[:, ct * 128:(ct + 1) * 128],
                            ident_bf[:],
                        )
                    nc.vector.tensor_copy(
                        xT[:, :, nt * 128:(nt + 1) * 128], ps_t[:]
                    )
                return u

            def emit_batch(b, carry):
                """Emit one batch; `carry` = proj closures of the previous
                batch, interleaved into this batch's preamble. Returns this
                batch's proj closures."""
                units = deque(carry)

                def drain(k=1):
                    for _ in range(k):
                        if units:
                            units.popleft()()

                if b > 0:
                    # prefix staging for this batch (casting gpsimd DMAs;
                    # the gpsimd engine reaches these while the previous
                    # attention still runs -> prefetch)
                    nc.gpsimd.dma_start(out=pkl[:], in_=pk_d[b])

                vb0 = v_units(b, 0)
                # the qk GEMM for token half jh only reads xT columns
                # jh*512..+512 (= x tiles 4jh..4jh+3), so half the qk
                # and v work starts after only FOUR tiles are
                # transposed -- the PE chews on it while tiles 4-7
                # stream in.
                qk0 = qk_units(b, 0)   # [k-jh0, k-jh1, q-jh0, q-jh1]
                qk1 = qk_units(b, 1)
                for nt in range(4):
                    tile_unit(b, nt)()
                    drain(1)
                for u in (qk0[0], qk0[2], qk1[0], qk1[2]):
                    u()
                    drain(1)
                for nt in range(4):
                    vb0[nt]()
                    if nt < 2:
                        tile_unit(b, 4 + nt)()
                    drain(1)
                tile_unit(b, 6)()
                tile_unit(b, 7)()
                # prefix: pk^T into kPre cols 0:16
                ps_pk = psG.tile([128, CT, P], BF16, tag="g",
                                 name=f"pspk_{b}")
                for ct in range(CT):
                    nc.tensor.transpose(
                        ps_pk[:, ct, :],
                        pkl[:, ct * 128:(ct + 1) * 128],
                        ident_bf[0:P, 0:P],
                    )
                nc.vector.tensor_copy(kPre[:, :, 0:P], ps_pk[:])
                if b > 0:
                    _pv_load(b)
                for u in (qk0[1], qk0[3], qk1[1], qk1[3]):
                    u()
                    drain(1)
                # NOTE: v_ext is a STATIONARY operand of the av matmuls;
                # writing it inside the consuming head's slots corrupts
                # (LDWEIGHTS pull-ahead loads stale data from within the
                # 64-instruction window, ignoring the semaphore order) --
                # measured rel-err 0.56/NaN.  Keep v block 0 fully in the
                # preamble.
                for nt in range(4, NT):
                    vb0[nt]()
                    drain(1)
                drain(len(units))  # force out any remaining carry

                def prefix_group(g):
                    """Packed prefix scores for heads 4g..4g+3 (pairs 2g,
                    2g+1): head h's 16 prefix keys land on psum rows
                    32*(h%4)..+32 (stationary is 32 wide, cols 16:32 zero),
                    so ONE exp serves 4 heads.  MMs are ordered row-half-
                    major so only verified-safe masked||masked overlap can
                    occur."""
                    ps_pre = psS.tile([128, N], F32, tag="s",
                                      name=f"pre_{b}_{g}")
                    for hh in (0, 1):
                        base = hh * 64
                        for hg in (hh, hh + 2):
                            h = 4 * g + hg
                            p = h // 2
                            for j in (0, 512):
                                nc.tensor.matmul(
                                    ps_pre[32 * hg:32 * hg + 32, j:j + 512],
                                    kPre[base:base + D, p, :],
                                    qT[base:base + D, p % 4, j:j + 512],
                                    start=True, stop=True,
                                    tile_position=(base, 32 * hg),
                                )
                    e_pre = epre_pool.tile([128, N], BF16, tag="ep",
                                           name=f"ep_{b}_{g}")
                    nc.scalar.activation(e_pre[:], ps_pre[:], AF.Exp,
                                         scale=SCALE)
                    return e_pre

                e_pre = prefix_group(0)

                # ---- per-head attention, gemm pipeline in the slots.
                # urgent = next-next pair's q/k (deadline: pair p+1 end);
                # lazy = v block 1 (deadline: pair 4) ----
                urgent = deque()
                lazy = deque()
                for p in range(HPAIRS):
                    if p + 2 < HPAIRS:
                        urgent.extend(qk_units(b, p + 2))
                    if p == 0:
                        lazy.extend(v_units(b, 1))
                    if p >= 2 and p % 2 == 0:
                        e_pre = prefix_group(p // 2)
                    lazy_budget = 2
                    slot = 0
                    for hh in range(2):
                        base = hh * 64
                        h = 2 * p + hh
                        ps_av = psAV.tile([128, N], F32, tag="av",
                                          name=f"av_{b}_{h}")
                        # prefix contribution from the shared packed exp
                        for j in (0, 512):
                            nc.tensor.matmul(
                                ps_av[:, j:j + 512],
                                v_ext[:, 0, h, :],
                                e_pre[:, j:j + 512],
                                start=True, stop=False,
                            )
                        for mt in range(1, MT):
                            ps_s = psS.tile([128, N], F32, tag="s",
                                            name=f"s_{b}_{h}_{mt}")
                            for j in (0, 512):
                                nc.tensor.matmul(
                                    ps_s[:, j:j + 512],
                                    kT[base:base + D, p % 4,
                                       (mt - 1) * 128:mt * 128],
                                    qT[base:base + D, p % 4, j:j + 512],
                                    start=True, stop=True,
                                )
                            eT = e_pool.tile([128, N], BF16, tag="e",
                                             name=f"e_{b}_{h}_{mt}")
                            nc.scalar.activation(eT[:], ps_s[:], AF.Exp,
                                                 scale=SCALE)
                            # gemm/proj filler BETWEEN exp and av: the PE
                            # would otherwise idle waiting for the exp (and,
                            # at mt==1, for the previous head's psum release)
                            slot += 1
                            if urgent and (mt == 5
                                           or len(urgent) >= 18 - slot):
                                urgent.popleft()()
                            elif lazy and lazy_budget > 0 and mt in (3, 7):
                                lazy.popleft()()
                                lazy_budget -= 1
                            for j in (0, 512):
                                nc.tensor.matmul(
                                    ps_av[:, j:j + 512],
                                    v_ext[:, mt, h, :],
                                    eT[:, j:j + 512],
                                    start=False, stop=(mt == MT - 1),
                                )
                        # normalize: out = unnorm * exp(-ln(denom)).
                        # (custom-DVE reciprocal_approx is unsupported by this
                        # walrus; iterative DVE reciprocal costs 6.5us.)
                        # The numerator is copied to SBUF so the psum
                        # accumulator is released after ~1.1us (copy || ln)
                        # instead of after the full ln->exp->mul chain.
                        num_sb = stg.tile([64, N], F32, tag="st",
                                          name=f"st_{b}_{h}")
                        nc.vector.tensor_copy(num_sb[:], ps_av[0:64, :])
                        lnd = rb_pool.tile([64, N], F32, tag="ln",
                                           name=f"ln_{b}_{h}")
                        nc.scalar.activation(lnd[:], ps_av[64:128, :], AF.Ln)
                        rb = rb_pool.tile([64, N], F32, tag="rb",
                                          name=f"rb_{b}_{h}")
                        nc.scalar.activation(rb[:], lnd[:], AF.Exp,
                                             scale=-1.0)
                        nc.vector.tensor_mul(
                            oT[base:base + D, p, :], num_sb[:], rb[:]
                        )
                        # head boundary: the next head's av-mt0 will block
                        # the in-order PE queue on the psAV release (the
                        # num copy above, ~1.2us) -- park a filler here
                        if urgent:
                            urgent.popleft()()
                        elif lazy:
                            lazy.popleft()()
                    if p >= HPAIRS - 3:
                        # tail: no further slots are guaranteed, flush
                        while urgent:
                            urgent.popleft()()
                        while lazy:
                            lazy.popleft()()

                return proj_units(b)

            carry = []
            for _rep in range(repeat):
                for b in range(B_PC):
                    carry = emit_batch(b, carry)
            for u in carry:
                u()

    return nc


_NC_CACHE = {}


def _get_nc(repeat: int = 1) -> bass.Bass:
    key = f"nc{repeat}"
    if key not in _NC_CACHE:
        _NC_CACHE[key] = build_nc(repeat)
    return _NC_CACHE[key]


def _make_runner(nc):
    """Compile the SPMD kernel ONCE into a reusable callable.

    Mirrors bass2jax.run_bass_via_pjrt's multi-core branch, but without
    output-buffer donation so the compiled function + device-resident
    inputs can be invoked repeatedly (for wall-clock benchmarking and to
    avoid recompiles on every kernel() call).
    """
    import jax
    from jax.experimental.shard_map import shard_map
    from jax.sharding import Mesh, PartitionSpec
    from concourse import bass2jax
    from concourse.bass2jax import _bass_exec_p, partition_id_tensor

    bass2jax.install_neuronx_cc_hook()

    partition_name = (
        nc.partition_id_tensor.name if nc.partition_id_tensor else None
    )
    in_names, out_names, out_avals, zero_outs = [], [], [], []
    for alloc in nc.m.functions[0].allocations:
        if not isinstance(alloc, mybir.MemoryLocationSet):
            continue
        name = alloc.memorylocations[0].name
        if alloc.kind == "ExternalInput":
            if name != partition_name:
                in_names.append(name)
        elif alloc.kind == "ExternalOutput":
            shape = tuple(alloc.tensor_shape)
            dtype = mybir.dt.np(alloc.dtype)
            out_names.append(name)
            out_avals.append(jax.core.ShapedArray(shape, dtype))
            zero_outs.append(np.zeros(shape, dtype))
    n_params = len(in_names)
    all_in_names = list(in_names) + list(out_names)
    if partition_name is not None:
        all_in_names.append(partition_name)

    def _body(*args):
        operands = list(args)
        if partition_name is not None:
            operands.append(partition_id_tensor())
        outs = _bass_exec_p.bind(
            *operands,
            out_avals=tuple(out_avals),
            in_names=tuple(all_in_names),
            out_names=tuple(out_names),
            lowering_input_output_aliases=(),
            sim_require_finite=True,
            sim_require_nnan=True,
            nc=nc,
        )
        return tuple(outs)

    devices = jax.devices()[:N_CORES]
    mesh = Mesh(np.asarray(devices), ("core",))
    n_outs = len(out_avals)
    in_specs = (PartitionSpec("core"),) * (n_params + n_outs)
    out_specs = (PartitionSpec("core"),) * n_outs
    sharded = jax.jit(
        shard_map(_body, mesh=mesh, in_specs=in_specs,
                  out_specs=out_specs, check_rep=False),
        keep_unused=True,
    )

    concat_zeros = [
        np.zeros((N_CORES * z.shape[0], *z.shape[1:]), z.dtype)
        for z in zero_outs
    ]

    state = {"dev_zeros": None}

    def runner(in_maps):
        per_core = [
            [np.asarray(m[name]) for name in in_names] for m in in_maps
        ]
        concat_in = [
            np.concatenate([per_core[c][i] for c in range(N_CORES)], axis=0)
            for i in range(n_params)
        ]
        if state["dev_zeros"] is None:
            state["dev_zeros"] = [jax.device_put(z) for z in concat_zeros]
        out_arrs = sharded(*concat_in, *state["dev_zeros"])
        return [
            {
                name: np.asarray(out_arrs[i]).reshape(
                    N_CORES, *out_avals[i].shape
                )[c]
                for i, name in enumerate(out_names)
            }
            for c in range(N_CORES)
        ]

    def runner_dev(dev_args):
        """dev_args: device-resident concat inputs; returns device outputs."""
        return sharded(*dev_args, *state["dev_zeros"])

    def make_dev_args(in_maps):
        per_core = [
            [np.asarray(m[name]) for name in in_names] for m in in_maps
        ]
        concat_in = [
            np.concatenate([per_core[c][i] for c in range(N_CORES)], axis=0)
            for i in range(n_params)
        ]
        if state["dev_zeros"] is None:
            state["dev_zeros"] = [jax.device_put(z) for z in concat_zeros]
        return [jax.device_put(a) for a in concat_in]

    return runner, runner_dev, make_dev_args


def _get_runner(repeat: int = 1):
    key = f"runner{repeat}"
    if key not in _NC_CACHE:
        _NC_CACHE[key] = _make_runner(_get_nc(repeat))
    return _NC_CACHE[key]


def _make_in_maps(x, pk, pv, w_qkv, w_proj, b_proj):
    x = np.ascontiguousarray(np.asarray(x, dtype=np.float32))
    pk = np.ascontiguousarray(np.asarray(pk, dtype=np.float32))
    pv = np.ascontiguousarray(np.asarray(pv, dtype=np.float32))
    w_qkv = np.ascontiguousarray(np.asarray(w_qkv, dtype=np.float32))
    w_proj = np.ascontiguousarray(np.asarray(w_proj, dtype=np.float32))
    b_proj = np.ascontiguousarray(np.asarray(b_proj, dtype=np.float32))
    in_maps = []
    for c in range(N_CORES):
        sl = slice(c * B_PC, (c + 1) * B_PC)
        in_maps.append({
            "x": x[sl], "pk": pk[sl], "pv": pv[sl],
            "w_qkv": w_qkv, "w_proj": w_proj, "b_proj": b_proj,
        })
    return in_maps


def run(x, pk, pv, w_qkv, w_proj, b_proj, trace=False, **trace_kwargs):
    """Run the SPMD kernel; returns (output [B,N,C], results).

    With trace=True, routes through run_bass_kernel_spmd so the returned
    results object carries .exec_time_ns / .profile_json.
    """
    in_maps = _make_in_maps(x, pk, pv, w_qkv, w_proj, b_proj)
    if trace:
        res = run_bass_kernel_spmd(
            _get_nc(), in_maps, list(range(N_CORES)), trace=True,
            **trace_kwargs,
        )
        results = res.results
        out = np.empty((B, N, C), dtype=np.float32)
        for c in range(N_CORES):
            outT = results[c]["outT"]          # [B_PC, C, N]
            out[c * B_PC:(c + 1) * B_PC] = outT.transpose(0, 2, 1)
        return out, res
    runner, _, _ = _get_runner()
    results = runner(in_maps)
    out = np.empty((B, N, C), dtype=np.float32)
    for c in range(N_CORES):
        outT = results[c]["outT"]              # [B_PC, C, N]
        out[c * B_PC:(c + 1) * B_PC] = outT.transpose(0, 2, 1)
    return out, results


def kernel(x, pk, pv, w_qkv, w_proj, b_proj) -> np.ndarray:
    out, _ = run(x, pk, pv, w_qkv, w_proj, b_proj)
    return out


def benchmark(x, pk, pv, w_qkv, w_proj, b_proj, iters=20, warmup=3, repeat=1):
    """Median wall-clock per executed call with device-resident inputs."""
    import time
    import jax
    _, runner_dev, make_dev_args = _get_runner(repeat)
    in_maps = _make_in_maps(x, pk, pv, w_qkv, w_proj, b_proj)
    dev_args = make_dev_args(in_maps)
    for _ in range(warmup):
        outs = runner_dev(dev_args)
        jax.block_until_ready(outs)
    ts = []
    for _ in range(iters):
        t0 = time.perf_counter()
        outs = runner_dev(dev_args)
        jax.block_until_ready(outs)
        ts.append(time.perf_counter() - t0)
    ts.sort()
    return {
        "median_s": ts[len(ts) // 2],
        "min_s": ts[0],
        "all_s": ts,
    }



# revision 49
# speedup vs baseline: 1.0142x; 1.0142x over previous
"""Trainium2 Bass kernel for prefix-KV multi-head attention (v3).

Reference computation (per batch):
    qkv = x @ w_qkv -> q,k,v heads; k/v get a 16-token prefix (pk, pv)
    attn = softmax(q @ k^T * D^-0.5); out = (attn @ v) @ w_proj + b_proj

Sharding: data-parallel over B across 8 NeuronCores (2 batches per core).

Design (v1 700us -> v2 617us -> v3 576us, all HW-measured):
  - weights resident in SBUF (bf16), loaded in 512-col chunks on the
    gpsimd sw-DGE queue (128-col chunks made 256B write packets and left
    the queue PACKET-RATE-bound ~60us; 1KB packets finish in ~25us),
    demand-ordered: pair-0/1 q/k, v block 0, pv, v block 1, rest, wproj
  - x rides the separate sync HW queue (4KB packets) concurrently,
    fp32 -> ACT cast -> bf16 PE transposes -> xT
  - preamble split by token half: the qk GEMM for half jh only reads
    x tiles 4jh..4jh+3, so qk/v GEMMs start after FOUR tiles instead of
    eight (PE chews while tiles 4-7 stream in)
  - a 32-MM warm-up burst at t=0 releases the HAM clock gate (PE is
    throttled to 1.2 GHz until ~3.4us of sustained activity)
  - qT/kT hold FOUR head pairs (slot p%4); pair p+2 is produced by
    pipelined fillers during pair p (urgent queue, 4 units/pair at mt
    slots 1/5); v block 1 + leftovers drain as lazy fillers (mt 3/7)
  - PACKED prefix: the 16 prefix keys of 4 heads land on 32-row stripes
    of ONE [128,1024] psum (stationary kPre is 32 wide with zero pad;
    explicit tile_position=(base,32*hg), row-half-major MM order so only
    verified-safe masked||masked overlap occurs) -> ONE exp serves 4
    heads instead of 4; v_ext m-tile 0 holds pv_h on partitions
    32*(h%4)..+16, zeros elsewhere, so other heads' e values in the
    shared e_pre contribute nothing
  - per-head attention over m-tiles 1..8 (tokens only): psS 2x2 banks
    double-buffered scores + psAV 2 banks av accumulator + psG 2x1
    gemm scratch = 8 banks exactly
  - ones-columns in v_ext give the softmax denominator for free
    (output ROWS of a matmul are free; cost = moving columns)
  - softmax 1/denominator via exp(-ln(d)) on ACT; a DVE copy of the
    numerator releases the av psum ~1.1us early
  - proj passes of the previous batch carry into the next preamble

Measured dead ends (don't retry without new evidence):
  - XBAR DMA transpose for x^T: 208B packets, 855us total (vs 576)
  - fold next batch's tiles/qk into this batch's tail pairs + split
    proj into ct-halves: 587-599us over four runs -- the attention
    phase is PE-saturated, so folded work stretches spans ~1:1 and the
    inter-batch "gap" was already dense PE work
  - x via casting gpsimd DMA (no fp32 staging): x cadence 2.3us/tile
    (vs 1.4 on sync queue) and it serializes behind weight chunks
  - DVE cast for xbf: DVE FIFO serializes with xT/qk copies, +7us
  - issuing av-mt0 after scores-mt1 (to hide the psAV-release wait):
    +9us regression
  - writing v_ext (a STATIONARY operand) inside the consuming head's
    slots: rel-err 0.56/NaN -- LDWEIGHTS pull-ahead (64-deep window)
    loads stale data, ignoring semaphore order; stationary operands
    must be written well outside the consumer's instruction window
  - v2's rejected list still stands: fp8 (5.7% err vs 2% tol), Pool
    normalize (742us), full score-pair row-packing with full-array av
    behind it (corrupts unless sync-guarded; guarded 641-759us)

Roofline notes (per core): PE ~540us active of 576 (94%); ACT exp
floor ~8.8us/head (1.18M elems / 128 lanes / 1.2GHz, dtype-blind) +
~2.3us/head normalize; per-head span ~12.4us vs ACT floor ~11.5 --
both engines are within ~10% of saturation, so further gains need the
score matmuls packed 2x (K=64) AND a cheaper normalize together.

Beware when benchmarking: after ~1h of sustained runs the chip enters
the P0 power state (PE 2.4 -> ~2.0 GHz; every engine ~19% slower, HAM
still shows K=8/8) and identical code measures ~688us instead of ~577.

This file is self-contained: it monkeypatches two workarounds for the
walrus build in this container (1-sync-wait-per-instruction cap).
"""

import json
import os
import sys
from collections import deque

for _p in ("/opt/trn_rl_repo", os.path.expanduser("~/.axon_site/_ro/trn_rl_repo")):
    if os.path.isdir(_p) and _p not in sys.path:
        sys.path.insert(0, _p)

import numpy as np

import concourse.bass as bass
import concourse.tile as tile
from concourse import mybir
from concourse.bass_utils import run_bass_kernel_spmd
from concourse.vector_clock import ScopedClock
from concourse.masks import make_identity

F32 = mybir.dt.float32
BF16 = mybir.dt.bfloat16
AF = mybir.ActivationFunctionType

# ---------------------------------------------------------------------------
# Workaround: this container's walrus supports at most ONE sync wait per
# instruction.  (a) split the TileContext-exit drain's waits onto single-wait
# NOPs; (b) at BIR-JSON serialization time, hoist extra waits from any
# instruction onto same-engine NOPs placed immediately before it.
# ---------------------------------------------------------------------------

def _patched_drain_and_barrier(self, tick_clock, wait_clock):
    drain_inst = self.nc.sync.drain()
    wait_clock.add_sem_waits(
        drain_inst.ins, ScopedClock({None: tick_clock.global_clock})
    )
    si = drain_inst.ins.sync_info
    waits = list(si.on_wait) if si is not None and si.on_wait else []
    if len(waits) > 1:
        si.on_wait = waits[:1]
        for w in waits[1:]:
            nop = self.nc.sync.nop(hint="drain_wait_split", nofuse=True)
            nsi = nop.ins.sync_info
            if nsi is None:
                nop.ins.sync_info = mybir.SyncInfo(on_wait=[w], on_update=[])
            else:
                nsi.on_wait = list(nsi.on_wait or []) + [w]
    self.nc.all_engine_barrier()
    assert self.sems is not None
    popped = self.nc._tile_sem_poison_stack.pop()
    assert popped is self._sem_poison
    self.nc.clear_and_free_semaphores(list(self.sems.allocated().values()))
    self.nc.all_engine_barrier()


tile.TileContext._drain_and_barrier = _patched_drain_and_barrier


def _split_multi_waits(bir):
    for fn in bir["functions"]:
        for bb in fn["blocks"]:
            new_insts = []
            for inst in bb["instructions"]:
                si = inst.get("sync_info")
                ow = (si or {}).get("on_wait") or []
                if len(ow) > 1:
                    for i, w in enumerate(ow[:-1]):
                        new_insts.append({
                            "debug": inst.get("debug", 0),
                            "engine": inst["engine"],
                            "ins": [], "outs": [],
                            "name": f"{inst['name']}.wsplit{i}",
                            "opcode": "NoOp",
                            "sync_info": {"on_wait": [w], "on_update": []},
                        })
                    si["on_wait"] = [ow[-1]]
                new_insts.append(inst)
            bb["instructions"] = new_insts
    return bir


_orig_to_json_bytes = bass.Bass.to_json_bytes


def _patched_to_json_bytes(self):
    d = json.loads(_orig_to_json_bytes(self))
    _split_multi_waits(d)
    return json.dumps(d).encode()


bass.Bass.to_json_bytes = _patched_to_json_bytes

# ---------------------------------------------------------------------------
# Problem constants (hardcoded per the task contract)
# ---------------------------------------------------------------------------

B, N, C, H, P = 16, 1024, 1024, 16, 16
D = C // H                      # 64
SCALE = float(D) ** -0.5        # 0.125
N_CORES = 8
B_PC = B // N_CORES             # 2 batches per core
NT = N // 128                   # 8 token tiles
CT = C // 128                   # 8 feature tiles
MT = NT + 1                     # 9 m-tiles: tile 0 = prefix (16 valid rows)
HPAIRS = H // 2                 # 8 head pairs
FOLD_NEXT = True


def build_nc(repeat: int = 1) -> bass.Bass:
    nc = bass.Bass()

    x_d = nc.declare_dram_parameter("x", [B_PC, N, C], F32, isOutput=False)
    pk_d = nc.declare_dram_parameter("pk", [B_PC, P, C], F32, isOutput=False)
    pv_d = nc.declare_dram_parameter("pv", [B_PC, P, C], F32, isOutput=False)
    wqkv_d = nc.declare_dram_parameter("w_qkv", [C, 3 * C], F32, isOutput=False)
    wproj_d = nc.declare_dram_parameter("w_proj", [C, C], F32, isOutput=False)
    bias_d = nc.declare_dram_parameter("b_proj", [C], F32, isOutput=False)
    # output is stored TRANSPOSED per batch: [C, N]; host transposes back
    outT_d = nc.declare_dram_parameter("outT", [B_PC, C, N], F32, isOutput=True)

    with tile.TileContext(nc) as tc:
        with tc.tile_pool(name="cons", bufs=1) as cons, \
             tc.tile_pool(name="eP", bufs=4) as e_pool, \
             tc.tile_pool(name="ePre", bufs=2) as epre_pool, \
             tc.tile_pool(name="stg", bufs=1) as stg, \
             tc.tile_pool(name="rbp", bufs=1) as rb_pool, \
             tc.tile_pool(name="xload", bufs=3) as xload, \
             tc.tile_pool(name="xbf", bufs=2) as xbfp, \
             tc.tile_pool(name="osb", bufs=2) as osb, \
             tc.tile_pool(name="psS", bufs=2, space="PSUM") as psS, \
             tc.tile_pool(name="psAV", bufs=1, space="PSUM") as psAV, \
             tc.tile_pool(name="psG", bufs=2, space="PSUM") as psG:

            # ---------------- one-time setup ----------------
            ident_bf = cons.tile([128, 128], BF16, tag="idb")
            make_identity(nc, ident_bf[:])
            # PE warm-up burst: ~3.5us of throwaway matmuls releases the
            # HAM clock-gate (K=4/8 -> 8/8) before the real work arrives,
            # so the preamble transposes/GEMMs run at 2.4 GHz not 1.2.
            warm_ps = psG.tile([128, 128], F32, tag="g", name="warmup")
            for _w in range(32):
                nc.tensor.matmul(
                    warm_ps[:], ident_bf[:], ident_bf[:],
                    start=(_w == 0), stop=(_w == 31),
                )
            # bias in per-partition layout: bias_col[p, cf] = b_proj[cf*128+p]
            bias_col = cons.tile([128, CT], F32, tag="bias")
            nc.sync.dma_start(
                out=bias_col[:],
                in_=bias_d[:].rearrange("(a b) -> b a", b=128),
            )
            # prefix-k staging (bf16 via casting gpsimd DMA)
            pkl = cons.tile([P, C], BF16, tag="pkl")

            # persistent activations (reused across batches; Tile tracks
            # read/write hazards on AP ranges).  qT/kT hold THREE head
            # pairs (slot p%3): pair p+2 is produced by pipelined fillers
            # while pair p's attention reads its slot; the extra slot lets
            # the packed-prefix exp (4 heads = 2 pairs per ACTIVATE) see
            # both of its pairs' q at group start.
            xT = cons.tile([128, CT, N], BF16, tag="xT")
            kT = cons.tile([128, 4, N], BF16, tag="kT")
            qT = cons.tile([128, 4, N], BF16, tag="qT")
            # prefix keys, all pairs: cols 0:16 = pk^T, 16:32 zero so the
            # packed 32-row score stripes come out 0 on rows 16:32 ->
            # exp = 1, harmless because the matching v_ext rows are zero
            kPre = cons.tile([128, HPAIRS, 32], BF16, tag="kPre")
            nc.vector.memset(kPre[:, :, P:32], 0.0)
            oT = cons.tile([128, CT, N], BF16, tag="oT")
            # v_ext[m, mt, h, 0:64] = v values; [.., 64:128] = ones columns
            # (denominator trick). m-tile 0 = prefix, PACKED: head h's 16
            # pv rows live at partitions 32*(h%4)..+16 (matching its stripe
            # in the packed prefix-score psum); all other rows stay ZERO so
            # the other heads' e values in the shared e_pre tile contribute
            # nothing to this head's av or denominator.
            v_ext = cons.tile([128, MT, H, 128], BF16, tag="vx")
            nc.vector.memset(v_ext[:, :, :, 64:128], 1.0)
            nc.vector.memset(v_ext[:, 0, :, :], 0.0)
            for a in range(4):
                nc.vector.memset(
                    v_ext[32 * a:32 * a + P, 0, a::4, 64:128], 1.0
                )

            # weights, bf16, resident for the whole kernel, on the gpsimd
            # sw-DGE queue (the only one that casts).  512-col chunks keep
            # the write packets at 1KB (128-col chunks made 256B packets and
            # left the queue packet-rate-bound for ~60us).  x rides the
            # separate sync HW queue concurrently.
            wq_sb = cons.tile([128, CT, C], BF16, tag="wq")
            wk_sb = cons.tile([128, CT, C], BF16, tag="wk")
            wv_sb = cons.tile([128, CT, C], BF16, tag="wv")
            wp_sb = cons.tile([128, CT, C], BF16, tag="wp")

            def _wload(dst, base, lo, hi):
                nc.gpsimd.dma_start(
                    out=dst[:, :, lo:hi],
                    in_=wqkv_d[:, base + lo:base + hi].rearrange(
                        "(ct p) f -> p ct f", p=128),
                )

            def _pv_load(b):
                pvr = pv_d[b].rearrange("t (h d) -> t h d", d=64)
                for a in range(4):
                    nc.gpsimd.dma_start(
                        out=v_ext[32 * a:32 * a + P, 0, a::4, 0:64],
                        in_=pvr[:, a::4, :],
                    )

            nc.gpsimd.dma_start(out=pkl[:], in_=pk_d[0])
            _wload(wk_sb, C, 0, 128)                  # k pair 0
            _wload(wq_sb, 0, 0, 128)                  # q pair 0
            _wload(wq_sb, 0, 128, 256)                # q pair 1
            _wload(wk_sb, C, 128, 256)                # k pair 1
            _wload(wv_sb, 2 * C, 0, 512)              # v block 0
            _pv_load(0)                               # prefix v, batch 0
            _wload(wv_sb, 2 * C, 512, 1024)           # v block 1
            _wload(wk_sb, C, 256, 640)
            _wload(wq_sb, 0, 256, 640)
            _wload(wk_sb, C, 640, 1024)
            _wload(wq_sb, 0, 640, 1024)
            nc.gpsimd.dma_start(
                out=wp_sb[:],
                in_=wproj_d[:].rearrange("(ct p) f -> p ct f", p=128),
            )

            # ---------------- per-batch work units ----------------

            def qk_units(b, p):
                """4 closures: q and k GEMMs for head pair p, split in two
                512-column halves each. Each accumulates 8 c-tiles into a
                [128,512] psum and copies (cast bf16) into qT/kT."""
                us = []
                for which in ("k", "q"):
                    for jh in range(2):
                        def u(which=which, p=p, jh=jh, b=b):
                            w_sb = wk_sb if which == "k" else wq_sb
                            ps = psG.tile([128, 512], F32, tag="g",
                                          name=f"g{which}_{b}_{p}_{jh}")
                            for ct in range(CT):
                                nc.tensor.matmul(
                                    ps[:],
                                    w_sb[:, ct, p * 128:(p + 1) * 128],
                                    xT[:, ct, jh * 512:(jh + 1) * 512],
                                    start=(ct == 0), stop=(ct == CT - 1),
                                )
                            if which == "k":
                                nc.vector.tensor_copy(
                                    kT[:, p % 4, jh * 512:(jh + 1) * 512],
                                    ps[:],
                                )
                            else:
                                nc.vector.tensor_copy(
                                    qT[:, p % 4, jh * 512:(jh + 1) * 512],
                                    ps[:],
                                )
                        us.append(u)
                return us

            def v_units(b, bk):
                """8 closures: v GEMM for pair block bk (4 pairs = 512 v
                columns), one per token tile. x^T tile is stationary, w_v
                columns are moving -> v lands in NATURAL [token, feature]
                layout, no transpose needed."""
                us = []
                for nt in range(NT):
                    def u(nt=nt, bk=bk, b=b):
                        ps = psG.tile([128, 512], F32, tag="g",
                                      name=f"gv_{b}_{bk}_{nt}")
                        for ct in range(CT):
                            nc.tensor.matmul(
                                ps[:],
                                xT[:, ct, nt * 128:(nt + 1) * 128],
                                wv_sb[:, ct, bk * 512:(bk + 1) * 512],
                                start=(ct == 0), stop=(ct == CT - 1),
                            )
                        nc.vector.tensor_copy(
                            v_ext[:, nt + 1, 8 * bk:8 * (bk + 1), 0:64],
                            ps[:].rearrange("p (h d) -> p h d", d=64),
                        )
                    us.append(u)
                return us

            def proj_units(b):
                """8 closures: one projection f-tile pass each; emitted
                interleaved into the NEXT batch's preamble."""
                us = []
                for cf in range(CT):
                    def u(cf=cf, b=b):
                        ps = psS.tile([128, N], F32, tag="s",
                                      name=f"pp_{b}_{cf}")
                        for ct in range(CT):
                            for j in (0, 512):
                                nc.tensor.matmul(
                                    ps[:, j:j + 512],
                                    wp_sb[:, ct, cf * 128:(cf + 1) * 128],
                                    oT[:, ct, j:j + 512],
                                    start=(ct == 0), stop=(ct == CT - 1),
                                )
                        o_sb = osb.tile([128, N], F32, tag="o",
                                        name=f"osb_{b}_{cf}")
                        nc.vector.tensor_scalar_add(
                            o_sb[:], ps[:], bias_col[:, cf:cf + 1]
                        )
                        nc.sync.dma_start(
                            out=outT_d[b, cf * 128:(cf + 1) * 128, :],
                            in_=o_sb[:],
                        )
                    us.append(u)
                return us

            def tile_unit(b, nt):
                """x tile -> bf16 -> x^T (sync-HW-queue DMA, ACT cast, PE
                transposes).  Returned as a closure so the NEXT batch's
                tiles can run as lazy filler in THIS batch's tail pairs.
                (XBAR DMA transpose was measured: 208B packets, 855us total
                -- the PE path is far faster for 128x128 tiles.)"""
                def u():
                    xl = xload.tile([128, C], F32, tag="xl",
                                    name=f"xl_{b}_{nt}")
                    nc.sync.dma_start(
                        out=xl[:], in_=x_d[b, nt * 128:(nt + 1) * 128, :]
                    )
                    xbf = xbfp.tile([128, C], BF16, tag="xbf",
                                    name=f"xbf_{b}_{nt}")
                    nc.scalar.activation(xbf[:], xl[:], AF.Copy)
                    ps_t = psG.tile([128, CT, 128], BF16, tag="g",
                                    name=f"pst_{b}_{nt}")
                    for ct in range(CT):
                        nc.tensor.transpose(
                            ps_t[:, ct, :],
                            xbf[:, ct * 128:(ct + 1) * 128],
                            ident_bf[:],
                        )
                    nc.vector.tensor_copy(
                        xT[:, :, nt * 128:(nt + 1) * 128], ps_t[:]
                    )
                return u

            def emit_batch(b, carry):
                """Emit one batch; `carry` = proj closures of the previous
                batch, interleaved into this batch's preamble. Returns this
                batch's proj closures."""
                units = deque(carry)

                def drain(k=1):
                    for _ in range(k):
                        if units:
                            units.popleft()()

                if b > 0:
                    # prefix staging for this batch (casting gpsimd DMAs;
                    # the gpsimd engine reaches these while the previous
                    # attention still runs -> prefetch)
                    nc.gpsimd.dma_start(out=pkl[:], in_=pk_d[b])

                vb0 = v_units(b, 0)
                # the qk GEMM for token half jh only reads xT columns
                # jh*512..+512 (= x tiles 4jh..4jh+3), so half the qk
                # and v work starts after only FOUR tiles are
                # transposed -- the PE chews on it while tiles 4-7
                # stream in.
                qk0 = qk_units(b, 0)   # [k-jh0, k-jh1, q-jh0, q-jh1]
                qk1 = qk_units(b, 1)
                for nt in range(4):
                    tile_unit(b, nt)()
                    drain(1)
                for u in (qk0[0], qk0[2], qk1[0], qk1[2]):
                    u()
                    drain(1)
                for nt in range(4):
                    vb0[nt]()
                    if nt < 2:
                        tile_unit(b, 4 + nt)()
                    drain(1)
                tile_unit(b, 6)()
                tile_unit(b, 7)()
                # prefix: pk^T into kPre cols 0:16
                ps_pk = psG.tile([128, CT, P], BF16, tag="g",
                                 name=f"pspk_{b}")
                for ct in range(CT):
                    nc.tensor.transpose(
                        ps_pk[:, ct, :],
                        pkl[:, ct * 128:(ct + 1) * 128],
                        ident_bf[0:P, 0:P],
                    )
                nc.vector.tensor_copy(kPre[:, :, 0:P], ps_pk[:])
                if b > 0:
                    _pv_load(b)
                for u in (qk0[1], qk0[3], qk1[1], qk1[3]):
                    u()
                    drain(1)
                # NOTE: v_ext is a STATIONARY operand of the av matmuls;
                # writing it inside the consuming head's slots corrupts
                # (LDWEIGHTS pull-ahead loads stale data from within the
                # 64-instruction window, ignoring the semaphore order) --
                # measured rel-err 0.56/NaN.  Keep v block 0 fully in the
                # preamble.
                for nt in range(4, NT):
                    vb0[nt]()
                    drain(1)
                drain(len(units))  # force out any remaining carry

                def prefix_group(g):
                    """Packed prefix scores for heads 4g..4g+3 (pairs 2g,
                    2g+1): head h's 16 prefix keys land on psum rows
                    32*(h%4)..+32 (stationary is 32 wide, cols 16:32 zero),
                    so ONE exp serves 4 heads.  MMs are ordered row-half-
                    major so only verified-safe masked||masked overlap can
                    occur."""
                    ps_pre = psS.tile([128, N], F32, tag="s",
                                      name=f"pre_{b}_{g}")
                    for hh in (0, 1):
                        base = hh * 64
                        for hg in (hh, hh + 2):
                            h = 4 * g + hg
                            p = h // 2
                            for j in (0, 512):
                                nc.tensor.matmul(
                                    ps_pre[32 * hg:32 * hg + 32, j:j + 512],
                                    kPre[base:base + D, p, :],
                                    qT[base:base + D, p % 4, j:j + 512],
                                    start=True, stop=True,
                                    tile_position=(base, 32 * hg),
                                )
                    e_pre = epre_pool.tile([128, N], BF16, tag="ep",
                                           name=f"ep_{b}_{g}")
                    nc.scalar.activation(e_pre[:], ps_pre[:], AF.Exp,
                                         scale=SCALE)
                    return e_pre

                e_pre = prefix_group(0)

                # ---- per-head attention, gemm pipeline in the slots.
                # urgent = next-next pair's q/k (deadline: pair p+1 end);
                # lazy = v block 1 (deadline: pair 4) ----
                urgent = deque()
                lazy = deque()
                for p in range(HPAIRS):
                    if p + 2 < HPAIRS:
                        urgent.extend(qk_units(b, p + 2))
                    if p == 0:
                        lazy.extend(v_units(b, 1))
                    if p >= 2 and p % 2 == 0:
                        e_pre = prefix_group(p // 2)
                    lazy_budget = 2
                    slot = 0
                    for hh in range(2):
                        base = hh * 64
                        h = 2 * p + hh
                        ps_av = psAV.tile([128, N], F32, tag="av",
                                          name=f"av_{b}_{h}")
                        # prefix contribution from the shared packed exp
                        for j in (0, 512):
                            nc.tensor.matmul(
                                ps_av[:, j:j + 512],
                                v_ext[:, 0, h, :],
                                e_pre[:, j:j + 512],
                                start=True, stop=False,
                            )
                        for mt in range(1, MT):
                            ps_s = psS.tile([128, N], F32, tag="s",
                                            name=f"s_{b}_{h}_{mt}")
                            for j in (0, 512):
                                nc.tensor.matmul(
                                    ps_s[:, j:j + 512],
                                    kT[base:base + D, p % 4,
                                       (mt - 1) * 128:mt * 128],
                                    qT[base:base + D, p % 4, j:j + 512],
                                    start=True, stop=True,
                                )
                            eT = e_pool.tile([128, N], BF16, tag="e",
                                             name=f"e_{b}_{h}_{mt}")
                            nc.scalar.activation(eT[:], ps_s[:], AF.Exp,
                                                 scale=SCALE)
                            # gemm/proj filler BETWEEN exp and av: the PE
                            # would otherwise idle waiting for the exp (and,
                            # at mt==1, for the previous head's psum release)
                            slot += 1
                            if urgent and (mt == 5
                                           or len(urgent) >= 18 - slot):
                                urgent.popleft()()
                            elif lazy and lazy_budget > 0 and mt in (3, 7):
                                lazy.popleft()()
                                lazy_budget -= 1
                            for j in (0, 512):
                                nc.tensor.matmul(
                                    ps_av[:, j:j + 512],
                                    v_ext[:, mt, h, :],
                                    eT[:, j:j + 512],
                                    start=False, stop=(mt == MT - 1),
                                )
                        # normalize: out = unnorm * exp(-ln(denom)).
                        # (custom-DVE reciprocal_approx is unsupported by this
                        # walrus; iterative DVE reciprocal costs 6.5us.)
                        # The numerator is copied to SBUF so the psum
                        # accumulator is released after ~1.1us (copy || ln)
                        # instead of after the full ln->exp->mul chain.
                        num_sb = stg.tile([64, N], F32, tag="st",
                                          name=f"st_{b}_{h}")
                        nc.vector.tensor_copy(num_sb[:], ps_av[0:64, :])
                        lnd = rb_pool.tile([64, N], F32, tag="ln",
                                           name=f"ln_{b}_{h}")
                        nc.scalar.activation(lnd[:], ps_av[64:128, :], AF.Ln)
                        rb = rb_pool.tile([64, N], F32, tag="rb",
                                          name=f"rb_{b}_{h}")
                        nc.scalar.activation(rb[:], lnd[:], AF.Exp,
                                             scale=-1.0)
                        nc.vector.tensor_mul(
                            oT[base:base + D, p, :], num_sb[:], rb[:]
                        )
                        # head boundary: the next head's av-mt0 will block
                        # the in-order PE queue on the psAV release (the
                        # num copy above, ~1.2us) -- park a filler here
                        if urgent:
                            urgent.popleft()()
                        elif lazy:
                            lazy.popleft()()
                    if p >= HPAIRS - 3:
                        # tail: no further slots are guaranteed, flush
                        while urgent:
                            urgent.popleft()()
                        while lazy:
                            lazy.popleft()()

                return proj_units(b)

            carry = []
            for _rep in range(repeat):
                for b in range(B_PC):
                    carry = emit_batch(b, carry)
            for u in carry:
                u()

    return nc


_NC_CACHE = {}


def _get_nc(repeat: int = 1) -> bass.Bass:
    key = f"nc{repeat}"
    if key not in _NC_CACHE:
        _NC_CACHE[key] = build_nc(repeat)
    return _NC_CACHE[key]


def _make_runner(nc):
    """Compile the SPMD kernel ONCE into a reusable callable.

    Mirrors bass2jax.run_bass_via_pjrt's multi-core branch, but without
    output-buffer donation so the compiled function + device-resident
    inputs can be invoked repeatedly (for wall-clock benchmarking and to
    avoid recompiles on every kernel() call).
    """
    import jax
    from jax.experimental.shard_map import shard_map
    from jax.sharding import Mesh, PartitionSpec
    from concourse import bass2jax
    from concourse.bass2jax import _bass_exec_p, partition_id_tensor

    bass2jax.install_neuronx_cc_hook()

    partition_name = (
        nc.partition_id_tensor.name if nc.partition_id_tensor else None
    )
    in_names, out_names, out_avals, zero_outs = [], [], [], []
    for alloc in nc.m.functions[0].allocations:
        if not isinstance(alloc, mybir.MemoryLocationSet):
            continue
        name = alloc.memorylocations[0].name
        if alloc.kind == "ExternalInput":
            if name != partition_name:
                in_names.append(name)
        elif alloc.kind == "ExternalOutput":
            shape = tuple(alloc.tensor_shape)
            dtype = mybir.dt.np(alloc.dtype)
            out_names.append(name)
            out_avals.append(jax.core.ShapedArray(shape, dtype))
            zero_outs.append(np.zeros(shape, dtype))
    n_params = len(in_names)
    all_in_names = list(in_names) + list(out_names)
    if partition_name is not None:
        all_in_names.append(partition_name)

    def _body(*args):
        operands = list(args)
        if partition_name is not None:
            operands.append(partition_id_tensor())
        outs = _bass_exec_p.bind(
            *operands,
            out_avals=tuple(out_avals),
            in_names=tuple(all_in_names),
            out_names=tuple(out_names),
            lowering_input_output_aliases=(),
            sim_require_finite=True,
            sim_require_nnan=True,
            nc=nc,
        )
        return tuple(outs)

    devices = jax.devices()[:N_CORES]
    mesh = Mesh(np.asarray(devices), ("core",))
    n_outs = len(out_avals)
    in_specs = (PartitionSpec("core"),) * (n_params + n_outs)
    out_specs = (PartitionSpec("core"),) * n_outs
    sharded = jax.jit(
        shard_map(_body, mesh=mesh, in_specs=in_specs,
                  out_specs=out_specs, check_rep=False),
        keep_unused=True,
    )

    concat_zeros = [
        np.zeros((N_CORES * z.shape[0], *z.shape[1:]), z.dtype)
        for z in zero_outs
    ]

    state = {"dev_zeros": None}

    def runner(in_maps):
        per_core = [
            [np.asarray(m[name]) for name in in_names] for m in in_maps
        ]
        concat_in = [
            np.concatenate([per_core[c][i] for c in range(N_CORES)], axis=0)
            for i in range(n_params)
        ]
        if state["dev_zeros"] is None:
            state["dev_zeros"] = [jax.device_put(z) for z in concat_zeros]
        out_arrs = sharded(*concat_in, *state["dev_zeros"])
        return [
            {
                name: np.asarray(out_arrs[i]).reshape(
                    N_CORES, *out_avals[i].shape
                )[c]
                for i, name in enumerate(out_names)
            }
            for c in range(N_CORES)
        ]

    def runner_dev(dev_args):
        """dev_args: device-resident concat inputs; returns device outputs."""
        return sharded(*dev_args, *state["dev_zeros"])

    def make_dev_args(in_maps):
        per_core = [
            [np.asarray(m[name]) for name in in_names] for m in in_maps
        ]
        concat_in = [
            np.concatenate([per_core[c][i] for c in range(N_CORES)], axis=0)
            for i in range(n_params)
        ]
        if state["dev_zeros"] is None:
            state["dev_zeros"] = [jax.device_put(z) for z in concat_zeros]
        return [jax.device_put(a) for a in concat_in]

    return runner, runner_dev, make_dev_args


def _get_runner(repeat: int = 1):
    key = f"runner{repeat}"
    if key not in _NC_CACHE:
        _NC_CACHE[key] = _make_runner(_get_nc(repeat))
    return _NC_CACHE[key]


def _make_in_maps(x, pk, pv, w_qkv, w_proj, b_proj):
    x = np.ascontiguousarray(np.asarray(x, dtype=np.float32))
    pk = np.ascontiguousarray(np.asarray(pk, dtype=np.float32))
    pv = np.ascontiguousarray(np.asarray(pv, dtype=np.float32))
    w_qkv = np.ascontiguousarray(np.asarray(w_qkv, dtype=np.float32))
    w_proj = np.ascontiguousarray(np.asarray(w_proj, dtype=np.float32))
    b_proj = np.ascontiguousarray(np.asarray(b_proj, dtype=np.float32))
    in_maps = []
    for c in range(N_CORES):
        sl = slice(c * B_PC, (c + 1) * B_PC)
        in_maps.append({
            "x": x[sl], "pk": pk[sl], "pv": pv[sl],
            "w_qkv": w_qkv, "w_proj": w_proj, "b_proj": b_proj,
        })
    return in_maps


def run(x, pk, pv, w_qkv, w_proj, b_proj, trace=False, **trace_kwargs):
    """Run the SPMD kernel; returns (output [B,N,C], results).

    With trace=True, routes through run_bass_kernel_spmd so the returned
    results object carries .exec_time_ns / .profile_json.
    """
    in_maps = _make_in_maps(x, pk, pv, w_qkv, w_proj, b_proj)
    if trace:
        res = run_bass_kernel_spmd(
            _get_nc(), in_maps, list(range(N_CORES)), trace=True,
            **trace_kwargs,
        )
        results = res.results
        out = np.empty((B, N, C), dtype=np.float32)
        for c in range(N_CORES):
            outT = results[c]["outT"]          # [B_PC, C, N]
            out[c * B_PC:(c + 1) * B_PC] = outT.transpose(0, 2, 1)
        return out, res
    runner, _, _ = _get_runner()
    results = runner(in_maps)
    out = np.empty((B, N, C), dtype=np.float32)
    for c in range(N_CORES):
        outT = results[c]["outT"]              # [B_PC, C, N]
        out[c * B_PC:(c + 1) * B_PC] = outT.transpose(0, 2, 1)
    return out, results


def kernel(x, pk, pv, w_qkv, w_proj, b_proj) -> np.ndarray:
    out, _ = run(x, pk, pv, w_qkv, w_proj, b_proj)
    return out


def benchmark(x, pk, pv, w_qkv, w_proj, b_proj, iters=20, warmup=3, repeat=1):
    """Median wall-clock per executed call with device-resident inputs."""
    import time
    import jax
    _, runner_dev, make_dev_args = _get_runner(repeat)
    in_maps = _make_in_maps(x, pk, pv, w_qkv, w_proj, b_proj)
    dev_args = make_dev_args(in_maps)
    for _ in range(warmup):
        outs = runner_dev(dev_args)
        jax.block_until_ready(outs)
    ts = []
    for _ in range(iters):
        t0 = time.perf_counter()
        outs = runner_dev(dev_args)
        jax.block_until_ready(outs)
        ts.append(time.perf_counter() - t0)
    ts.sort()
    return {
        "median_s": ts[len(ts) // 2],
        "min_s": ts[0],
        "all_s": ts,
    }



# revision 50
# speedup vs baseline: 1.0159x; 1.0017x over previous
"""Trainium2 Bass kernel for prefix-KV multi-head attention (v3).

Reference computation (per batch):
    qkv = x @ w_qkv -> q,k,v heads; k/v get a 16-token prefix (pk, pv)
    attn = softmax(q @ k^T * D^-0.5); out = (attn @ v) @ w_proj + b_proj

Sharding: data-parallel over B across 8 NeuronCores (2 batches per core).

Design (v1 700us -> v2 617us -> v3 576us, all HW-measured):
  - weights resident in SBUF (bf16), loaded in 512-col chunks on the
    gpsimd sw-DGE queue (128-col chunks made 256B write packets and left
    the queue PACKET-RATE-bound ~60us; 1KB packets finish in ~25us),
    demand-ordered: pair-0/1 q/k, v block 0, pv, v block 1, rest, wproj
  - x rides the separate sync HW queue (4KB packets) concurrently,
    fp32 -> ACT cast -> bf16 PE transposes -> xT
  - preamble split by token half: the qk GEMM for half jh only reads
    x tiles 4jh..4jh+3, so qk/v GEMMs start after FOUR tiles instead of
    eight (PE chews while tiles 4-7 stream in)
  - a 32-MM warm-up burst at t=0 releases the HAM clock gate (PE is
    throttled to 1.2 GHz until ~3.4us of sustained activity)
  - qT/kT hold FOUR head pairs (slot p%4); pair p+2 is produced by
    pipelined fillers during pair p (urgent queue, 4 units/pair at mt
    slots 1/5); v block 1 + leftovers drain as lazy fillers (mt 3/7)
  - PACKED prefix: the 16 prefix keys of 4 heads land on 32-row stripes
    of ONE [128,1024] psum (stationary kPre is 32 wide with zero pad;
    explicit tile_position=(base,32*hg), row-half-major MM order so only
    verified-safe masked||masked overlap occurs) -> ONE exp serves 4
    heads instead of 4; v_ext m-tile 0 holds pv_h on partitions
    32*(h%4)..+16, zeros elsewhere, so other heads' e values in the
    shared e_pre contribute nothing
  - per-head attention over m-tiles 1..8 (tokens only): psS 2x2 banks
    double-buffered scores + psAV 2 banks av accumulator + psG 2x1
    gemm scratch = 8 banks exactly
  - ones-columns in v_ext give the softmax denominator for free
    (output ROWS of a matmul are free; cost = moving columns)
  - softmax 1/denominator via exp(-ln(d)) on ACT; a DVE copy of the
    numerator releases the av psum ~1.1us early
  - proj passes of the previous batch carry into the next preamble

Measured dead ends (don't retry without new evidence):
  - XBAR DMA transpose for x^T: 208B packets, 855us total (vs 576)
  - fold next batch's tiles/qk into this batch's tail pairs + split
    proj into ct-halves: 587-599us over four runs -- the attention
    phase is PE-saturated, so folded work stretches spans ~1:1 and the
    inter-batch "gap" was already dense PE work
  - x via casting gpsimd DMA (no fp32 staging): x cadence 2.3us/tile
    (vs 1.4 on sync queue) and it serializes behind weight chunks
  - DVE cast for xbf: DVE FIFO serializes with xT/qk copies, +7us
  - issuing av-mt0 after scores-mt1 (to hide the psAV-release wait):
    +9us regression
  - writing v_ext (a STATIONARY operand) inside the consuming head's
    slots: rel-err 0.56/NaN -- LDWEIGHTS pull-ahead (64-deep window)
    loads stale data, ignoring semaphore order; stationary operands
    must be written well outside the consumer's instruction window
  - v2's rejected list still stands: fp8 (5.7% err vs 2% tol), Pool
    normalize (742us), full score-pair row-packing with full-array av
    behind it (corrupts unless sync-guarded; guarded 641-759us)

Roofline notes (per core): PE ~540us active of 576 (94%); ACT exp
floor ~8.8us/head (1.18M elems / 128 lanes / 1.2GHz, dtype-blind) +
~2.3us/head normalize; per-head span ~12.4us vs ACT floor ~11.5 --
both engines are within ~10% of saturation, so further gains need the
score matmuls packed 2x (K=64) AND a cheaper normalize together.

Beware when benchmarking: after ~1h of sustained runs the chip enters
the P0 power state (PE 2.4 -> ~2.0 GHz; every engine ~19% slower, HAM
still shows K=8/8) and identical code measures ~688us instead of ~577.

This file is self-contained: it monkeypatches two workarounds for the
walrus build in this container (1-sync-wait-per-instruction cap).
"""

import json
import os
import sys
from collections import deque

for _p in ("/opt/trn_rl_repo", os.path.expanduser("~/.axon_site/_ro/trn_rl_repo")):
    if os.path.isdir(_p) and _p not in sys.path:
        sys.path.insert(0, _p)

import numpy as np

import concourse.bass as bass
import concourse.tile as tile
from concourse import mybir
from concourse.bass_utils import run_bass_kernel_spmd
from concourse.vector_clock import ScopedClock
from concourse.masks import make_identity

F32 = mybir.dt.float32
BF16 = mybir.dt.bfloat16
AF = mybir.ActivationFunctionType

# ---------------------------------------------------------------------------
# Workaround: this container's walrus supports at most ONE sync wait per
# instruction.  (a) split the TileContext-exit drain's waits onto single-wait
# NOPs; (b) at BIR-JSON serialization time, hoist extra waits from any
# instruction onto same-engine NOPs placed immediately before it.
# ---------------------------------------------------------------------------

def _patched_drain_and_barrier(self, tick_clock, wait_clock):
    drain_inst = self.nc.sync.drain()
    wait_clock.add_sem_waits(
        drain_inst.ins, ScopedClock({None: tick_clock.global_clock})
    )
    si = drain_inst.ins.sync_info
    waits = list(si.on_wait) if si is not None and si.on_wait else []
    if len(waits) > 1:
        si.on_wait = waits[:1]
        for w in waits[1:]:
            nop = self.nc.sync.nop(hint="drain_wait_split", nofuse=True)
            nsi = nop.ins.sync_info
            if nsi is None:
                nop.ins.sync_info = mybir.SyncInfo(on_wait=[w], on_update=[])
            else:
                nsi.on_wait = list(nsi.on_wait or []) + [w]
    self.nc.all_engine_barrier()
    assert self.sems is not None
    popped = self.nc._tile_sem_poison_stack.pop()
    assert popped is self._sem_poison
    self.nc.clear_and_free_semaphores(list(self.sems.allocated().values()))
    self.nc.all_engine_barrier()


tile.TileContext._drain_and_barrier = _patched_drain_and_barrier


def _split_multi_waits(bir):
    for fn in bir["functions"]:
        for bb in fn["blocks"]:
            new_insts = []
            for inst in bb["instructions"]:
                si = inst.get("sync_info")
                ow = (si or {}).get("on_wait") or []
                if len(ow) > 1:
                    for i, w in enumerate(ow[:-1]):
                        new_insts.append({
                            "debug": inst.get("debug", 0),
                            "engine": inst["engine"],
                            "ins": [], "outs": [],
                            "name": f"{inst['name']}.wsplit{i}",
                            "opcode": "NoOp",
                            "sync_info": {"on_wait": [w], "on_update": []},
                        })
                    si["on_wait"] = [ow[-1]]
                new_insts.append(inst)
            bb["instructions"] = new_insts
    return bir


_orig_to_json_bytes = bass.Bass.to_json_bytes


def _patched_to_json_bytes(self):
    d = json.loads(_orig_to_json_bytes(self))
    _split_multi_waits(d)
    return json.dumps(d).encode()


bass.Bass.to_json_bytes = _patched_to_json_bytes

# ---------------------------------------------------------------------------
# Problem constants (hardcoded per the task contract)
# ---------------------------------------------------------------------------

B, N, C, H, P = 16, 1024, 1024, 16, 16
D = C // H                      # 64
SCALE = float(D) ** -0.5        # 0.125
N_CORES = 8
B_PC = B // N_CORES             # 2 batches per core
NT = N // 128                   # 8 token tiles
CT = C // 128                   # 8 feature tiles
MT = NT + 1                     # 9 m-tiles: tile 0 = prefix (16 valid rows)
HPAIRS = H // 2                 # 8 head pairs
FOLD_NEXT = True


def build_nc(repeat: int = 1) -> bass.Bass:
    nc = bass.Bass()

    x_d = nc.declare_dram_parameter("x", [B_PC, N, C], F32, isOutput=False)
    pk_d = nc.declare_dram_parameter("pk", [B_PC, P, C], F32, isOutput=False)
    pv_d = nc.declare_dram_parameter("pv", [B_PC, P, C], F32, isOutput=False)
    wqkv_d = nc.declare_dram_parameter("w_qkv", [C, 3 * C], F32, isOutput=False)
    wproj_d = nc.declare_dram_parameter("w_proj", [C, C], F32, isOutput=False)
    bias_d = nc.declare_dram_parameter("b_proj", [C], F32, isOutput=False)
    # output is stored TRANSPOSED per batch: [C, N]; host transposes back
    outT_d = nc.declare_dram_parameter("outT", [B_PC, C, N], F32, isOutput=True)

    with tile.TileContext(nc) as tc:
        with tc.tile_pool(name="cons", bufs=1) as cons, \
             tc.tile_pool(name="eP", bufs=4) as e_pool, \
             tc.tile_pool(name="ePre", bufs=2) as epre_pool, \
             tc.tile_pool(name="stg", bufs=1) as stg, \
             tc.tile_pool(name="rbp", bufs=1) as rb_pool, \
             tc.tile_pool(name="xload", bufs=3) as xload, \
             tc.tile_pool(name="xbf", bufs=2) as xbfp, \
             tc.tile_pool(name="osb", bufs=2) as osb, \
             tc.tile_pool(name="psS", bufs=2, space="PSUM") as psS, \
             tc.tile_pool(name="psAV", bufs=1, space="PSUM") as psAV, \
             tc.tile_pool(name="psG", bufs=2, space="PSUM") as psG:

            # ---------------- one-time setup ----------------
            ident_bf = cons.tile([128, 128], BF16, tag="idb")
            make_identity(nc, ident_bf[:])
            # PE warm-up burst: ~3.5us of throwaway matmuls releases the
            # HAM clock-gate (K=4/8 -> 8/8) before the real work arrives,
            # so the preamble transposes/GEMMs run at 2.4 GHz not 1.2.
            warm_ps = psG.tile([128, 128], F32, tag="g", name="warmup")
            for _w in range(32):
                nc.tensor.matmul(
                    warm_ps[:], ident_bf[:], ident_bf[:],
                    start=(_w == 0), stop=(_w == 31),
                )
            # bias in per-partition layout: bias_col[p, cf] = b_proj[cf*128+p]
            bias_col = cons.tile([128, CT], F32, tag="bias")
            nc.sync.dma_start(
                out=bias_col[:],
                in_=bias_d[:].rearrange("(a b) -> b a", b=128),
            )
            # prefix-k staging (bf16 via casting gpsimd DMA)
            pkl = cons.tile([P, C], BF16, tag="pkl")

            # persistent activations (reused across batches; Tile tracks
            # read/write hazards on AP ranges).  qT/kT hold THREE head
            # pairs (slot p%3): pair p+2 is produced by pipelined fillers
            # while pair p's attention reads its slot; the extra slot lets
            # the packed-prefix exp (4 heads = 2 pairs per ACTIVATE) see
            # both of its pairs' q at group start.
            xT = cons.tile([128, CT, N], BF16, tag="xT")
            kT = cons.tile([128, 4, N], BF16, tag="kT")
            qT = cons.tile([128, 4, N], BF16, tag="qT")
            # prefix keys, all pairs: cols 0:16 = pk^T, 16:32 zero so the
            # packed 32-row score stripes come out 0 on rows 16:32 ->
            # exp = 1, harmless because the matching v_ext rows are zero
            kPre = cons.tile([128, HPAIRS, 32], BF16, tag="kPre")
            nc.vector.memset(kPre[:, :, P:32], 0.0)
            oT = cons.tile([128, CT, N], BF16, tag="oT")
            # v_ext[m, mt, h, 0:64] = v values; [.., 64:128] = ones columns
            # (denominator trick). m-tile 0 = prefix, PACKED: head h's 16
            # pv rows live at partitions 32*(h%4)..+16 (matching its stripe
            # in the packed prefix-score psum); all other rows stay ZERO so
            # the other heads' e values in the shared e_pre tile contribute
            # nothing to this head's av or denominator.
            v_ext = cons.tile([128, MT, H, 128], BF16, tag="vx")
            nc.vector.memset(v_ext[:, :, :, 64:128], 1.0)
            nc.vector.memset(v_ext[:, 0, :, :], 0.0)
            for a in range(4):
                nc.vector.memset(
                    v_ext[32 * a:32 * a + P, 0, a::4, 64:128], 1.0
                )

            # weights, bf16, resident for the whole kernel, on the gpsimd
            # sw-DGE queue (the only one that casts).  512-col chunks keep
            # the write packets at 1KB (128-col chunks made 256B packets and
            # left the queue packet-rate-bound for ~60us).  x rides the
            # separate sync HW queue concurrently.
            wq_sb = cons.tile([128, CT, C], BF16, tag="wq")
            wk_sb = cons.tile([128, CT, C], BF16, tag="wk")
            wv_sb = cons.tile([128, CT, C], BF16, tag="wv")
            wp_sb = cons.tile([128, CT, C], BF16, tag="wp")

            def _wload(dst, base, lo, hi):
                nc.gpsimd.dma_start(
                    out=dst[:, :, lo:hi],
                    in_=wqkv_d[:, base + lo:base + hi].rearrange(
                        "(ct p) f -> p ct f", p=128),
                )

            def _pv_load(b):
                pvr = pv_d[b].rearrange("t (h d) -> t h d", d=64)
                for a in range(4):
                    nc.gpsimd.dma_start(
                        out=v_ext[32 * a:32 * a + P, 0, a::4, 0:64],
                        in_=pvr[:, a::4, :],
                    )

            nc.gpsimd.dma_start(out=pkl[:], in_=pk_d[0])
            _wload(wk_sb, C, 0, 128)                  # k pair 0
            _wload(wq_sb, 0, 0, 128)                  # q pair 0
            _wload(wq_sb, 0, 128, 256)                # q pair 1
            _wload(wk_sb, C, 128, 256)                # k pair 1
            _wload(wv_sb, 2 * C, 0, 512)              # v block 0
            _pv_load(0)                               # prefix v, batch 0
            _wload(wv_sb, 2 * C, 512, 1024)           # v block 1
            _wload(wk_sb, C, 256, 640)
            _wload(wq_sb, 0, 256, 640)
            _wload(wk_sb, C, 640, 1024)
            _wload(wq_sb, 0, 640, 1024)
            nc.gpsimd.dma_start(
                out=wp_sb[:],
                in_=wproj_d[:].rearrange("(ct p) f -> p ct f", p=128),
            )

            # ---------------- per-batch work units ----------------

            def qk_units(b, p):
                """4 closures: q and k GEMMs for head pair p, split in two
                512-column halves each. Each accumulates 8 c-tiles into a
                [128,512] psum and copies (cast bf16) into qT/kT."""
                us = []
                for which in ("k", "q"):
                    for jh in range(2):
                        def u(which=which, p=p, jh=jh, b=b):
                            w_sb = wk_sb if which == "k" else wq_sb
                            ps = psG.tile([128, 512], F32, tag="g",
                                          name=f"g{which}_{b}_{p}_{jh}")
                            for ct in range(CT):
                                nc.tensor.matmul(
                                    ps[:],
                                    w_sb[:, ct, p * 128:(p + 1) * 128],
                                    xT[:, ct, jh * 512:(jh + 1) * 512],
                                    start=(ct == 0), stop=(ct == CT - 1),
                                )
                            if which == "k":
                                nc.vector.tensor_copy(
                                    kT[:, p % 4, jh * 512:(jh + 1) * 512],
                                    ps[:],
                                )
                            else:
                                nc.vector.tensor_copy(
                                    qT[:, p % 4, jh * 512:(jh + 1) * 512],
                                    ps[:],
                                )
                        us.append(u)
                return us

            def v_units(b, bk):
                """8 closures: v GEMM for pair block bk (4 pairs = 512 v
                columns), one per token tile. x^T tile is stationary, w_v
                columns are moving -> v lands in NATURAL [token, feature]
                layout, no transpose needed."""
                us = []
                for nt in range(NT):
                    def u(nt=nt, bk=bk, b=b):
                        ps = psG.tile([128, 512], F32, tag="g",
                                      name=f"gv_{b}_{bk}_{nt}")
                        for ct in range(CT):
                            nc.tensor.matmul(
                                ps[:],
                                xT[:, ct, nt * 128:(nt + 1) * 128],
                                wv_sb[:, ct, bk * 512:(bk + 1) * 512],
                                start=(ct == 0), stop=(ct == CT - 1),
                            )
                        nc.vector.tensor_copy(
                            v_ext[:, nt + 1, 8 * bk:8 * (bk + 1), 0:64],
                            ps[:].rearrange("p (h d) -> p h d", d=64),
                        )
                    us.append(u)
                return us

            def proj_units(b):
                """8 closures: one projection f-tile pass each; emitted
                interleaved into the NEXT batch's preamble."""
                us = []
                for cf in range(CT):
                    def u(cf=cf, b=b):
                        ps = psS.tile([128, N], F32, tag="s",
                                      name=f"pp_{b}_{cf}")
                        for ct in range(CT):
                            for j in (0, 512):
                                nc.tensor.matmul(
                                    ps[:, j:j + 512],
                                    wp_sb[:, ct, cf * 128:(cf + 1) * 128],
                                    oT[:, ct, j:j + 512],
                                    start=(ct == 0), stop=(ct == CT - 1),
                                )
                        o_sb = osb.tile([128, N], F32, tag="o",
                                        name=f"osb_{b}_{cf}")
                        nc.vector.tensor_scalar_add(
                            o_sb[:], ps[:], bias_col[:, cf:cf + 1]
                        )
                        nc.sync.dma_start(
                            out=outT_d[b, cf * 128:(cf + 1) * 128, :],
                            in_=o_sb[:],
                        )
                    us.append(u)
                return us

            def tile_unit(b, nt):
                """x tile -> bf16 -> x^T (sync-HW-queue DMA, ACT cast, PE
                transposes).  Returned as a closure so the NEXT batch's
                tiles can run as lazy filler in THIS batch's tail pairs.
                (XBAR DMA transpose was measured: 208B packets, 855us total
                -- the PE path is far faster for 128x128 tiles.)"""
                def u():
                    xl = xload.tile([128, C], F32, tag="xl",
                                    name=f"xl_{b}_{nt}")
                    nc.sync.dma_start(
                        out=xl[:], in_=x_d[b, nt * 128:(nt + 1) * 128, :]
                    )
                    xbf = xbfp.tile([128, C], BF16, tag="xbf",
                                    name=f"xbf_{b}_{nt}")
                    nc.scalar.activation(xbf[:], xl[:], AF.Copy)
                    ps_t = psG.tile([128, CT, 128], BF16, tag="g",
                                    name=f"pst_{b}_{nt}")
                    for ct in range(CT):
                        nc.tensor.transpose(
                            ps_t[:, ct, :],
                            xbf[:, ct * 128:(ct + 1) * 128],
                            ident_bf[:],
                        )
                    nc.vector.tensor_copy(
                        xT[:, :, nt * 128:(nt + 1) * 128], ps_t[:]
                    )
                return u

            def emit_batch(b, carry):
                """Emit one batch; `carry` = proj closures of the previous
                batch, interleaved into this batch's preamble. Returns this
                batch's proj closures."""
                units = deque(carry)

                def drain(k=1):
                    for _ in range(k):
                        if units:
                            units.popleft()()

                if b > 0:
                    # prefix staging for this batch (casting gpsimd DMAs;
                    # the gpsimd engine reaches these while the previous
                    # attention still runs -> prefetch)
                    nc.gpsimd.dma_start(out=pkl[:], in_=pk_d[b])

                vb0 = v_units(b, 0)
                # the qk GEMM for token half jh only reads xT columns
                # jh*512..+512 (= x tiles 4jh..4jh+3), so half the qk
                # and v work starts after only FOUR tiles are
                # transposed -- the PE chews on it while tiles 4-7
                # stream in.
                qk0 = qk_units(b, 0)   # [k-jh0, k-jh1, q-jh0, q-jh1]
                qk1 = qk_units(b, 1)
                for nt in range(4):
                    tile_unit(b, nt)()
                    drain(1)
                for u in (qk0[0], qk0[2], qk1[0], qk1[2]):
                    u()
                    drain(1)
                for nt in range(4):
                    vb0[nt]()
                    if nt < 2:
                        tile_unit(b, 4 + nt)()
                    drain(1)
                tile_unit(b, 6)()
                tile_unit(b, 7)()
                # prefix: pk^T into kPre cols 0:16
                ps_pk = psG.tile([128, CT, P], BF16, tag="g",
                                 name=f"pspk_{b}")
                for ct in range(CT):
                    nc.tensor.transpose(
                        ps_pk[:, ct, :],
                        pkl[:, ct * 128:(ct + 1) * 128],
                        ident_bf[0:P, 0:P],
                    )
                nc.vector.tensor_copy(kPre[:, :, 0:P], ps_pk[:])
                if b > 0:
                    _pv_load(b)
                for u in (qk0[1], qk0[3], qk1[1], qk1[3]):
                    u()
                    drain(1)
                # NOTE: v_ext is a STATIONARY operand of the av matmuls;
                # writing it inside the consuming head's slots corrupts
                # (LDWEIGHTS pull-ahead loads stale data from within the
                # 64-instruction window, ignoring the semaphore order) --
                # measured rel-err 0.56/NaN.  Keep v block 0 fully in the
                # preamble.
                for nt in range(4, NT):
                    vb0[nt]()
                    drain(1)
                drain(len(units))  # force out any remaining carry

                def prefix_group(g):
                    """Packed prefix scores for heads 4g..4g+3 (pairs 2g,
                    2g+1): head h's 16 prefix keys land on psum rows
                    32*(h%4)..+32 (stationary is 32 wide, cols 16:32 zero),
                    so ONE exp serves 4 heads.  MMs are ordered row-half-
                    major so only verified-safe masked||masked overlap can
                    occur."""
                    ps_pre = psS.tile([128, N], F32, tag="s",
                                      name=f"pre_{b}_{g}")
                    # row-ALTERNATING order (0,64,0,64...): every
                    # adjacent MM differs in row_grp, so LDWEIGHTS loads
                    # during the previous MM and the masked MMs overlap
                    # pairwise (the verified-safe masked||masked case).
                    # The group is followed only by masked scores MMs;
                    # the full-array av waits on this group's exp, so it
                    # cannot enter the array while these are in flight.
                    for j in (0, 512):
                        for hg in range(4):
                            h = 4 * g + hg
                            base = (hg % 2) * 64
                            p = h // 2
                            nc.tensor.matmul(
                                ps_pre[32 * hg:32 * hg + 32, j:j + 512],
                                kPre[base:base + D, p, :],
                                qT[base:base + D, p % 4, j:j + 512],
                                start=True, stop=True,
                                tile_position=(base, 32 * hg),
                            )
                    e_pre = epre_pool.tile([128, N], BF16, tag="ep",
                                           name=f"ep_{b}_{g}")
                    nc.scalar.activation(e_pre[:], ps_pre[:], AF.Exp,
                                         scale=SCALE)
                    return e_pre

                e_pre = prefix_group(0)

                # ---- per-head attention, gemm pipeline in the slots.
                # urgent = next-next pair's q/k (deadline: pair p+1 end);
                # lazy = v block 1 (deadline: pair 4) ----
                urgent = deque()
                lazy = deque()
                for p in range(HPAIRS):
                    if p + 2 < HPAIRS:
                        urgent.extend(qk_units(b, p + 2))
                    if p == 0:
                        lazy.extend(v_units(b, 1))
                    if p >= 2 and p % 2 == 0:
                        e_pre = prefix_group(p // 2)
                    lazy_budget = 2
                    slot = 0
                    for hh in range(2):
                        base = hh * 64
                        h = 2 * p + hh
                        ps_av = psAV.tile([128, N], F32, tag="av",
                                          name=f"av_{b}_{h}")
                        # prefix contribution from the shared packed exp
                        for j in (0, 512):
                            nc.tensor.matmul(
                                ps_av[:, j:j + 512],
                                v_ext[:, 0, h, :],
                                e_pre[:, j:j + 512],
                                start=True, stop=False,
                            )
                        for mt in range(1, MT):
                            ps_s = psS.tile([128, N], F32, tag="s",
                                            name=f"s_{b}_{h}_{mt}")
                            for j in (0, 512):
                                nc.tensor.matmul(
                                    ps_s[:, j:j + 512],
                                    kT[base:base + D, p % 4,
                                       (mt - 1) * 128:mt * 128],
                                    qT[base:base + D, p % 4, j:j + 512],
                                    start=True, stop=True,
                                )
                            eT = e_pool.tile([128, N], BF16, tag="e",
                                             name=f"e_{b}_{h}_{mt}")
                            nc.scalar.activation(eT[:], ps_s[:], AF.Exp,
                                                 scale=SCALE)
                            # gemm/proj filler BETWEEN exp and av: the PE
                            # would otherwise idle waiting for the exp (and,
                            # at mt==1, for the previous head's psum release)
                            slot += 1
                            if urgent and (mt == 5
                                           or len(urgent) >= 18 - slot):
                                urgent.popleft()()
                            elif lazy and lazy_budget > 0 and mt in (3, 7):
                                lazy.popleft()()
                                lazy_budget -= 1
                            for j in (0, 512):
                                nc.tensor.matmul(
                                    ps_av[:, j:j + 512],
                                    v_ext[:, mt, h, :],
                                    eT[:, j:j + 512],
                                    start=False, stop=(mt == MT - 1),
                                )
                        # normalize: out = unnorm * exp(-ln(denom)).
                        # (custom-DVE reciprocal_approx is unsupported by this
                        # walrus; iterative DVE reciprocal costs 6.5us.)
                        # The numerator is copied to SBUF so the psum
                        # accumulator is released after ~1.1us (copy || ln)
                        # instead of after the full ln->exp->mul chain.
                        num_sb = stg.tile([64, N], F32, tag="st",
                                          name=f"st_{b}_{h}")
                        nc.vector.tensor_copy(num_sb[:], ps_av[0:64, :])
                        lnd = rb_pool.tile([64, N], F32, tag="ln",
                                           name=f"ln_{b}_{h}")
                        nc.scalar.activation(lnd[:], ps_av[64:128, :], AF.Ln)
                        rb = rb_pool.tile([64, N], F32, tag="rb",
                                          name=f"rb_{b}_{h}")
                        nc.scalar.activation(rb[:], lnd[:], AF.Exp,
                                             scale=-1.0)
                        nc.vector.tensor_mul(
                            oT[base:base + D, p, :], num_sb[:], rb[:]
                        )
                        # head boundary: the next head's av-mt0 will block
                        # the in-order PE queue on the psAV release (the
                        # num copy above, ~1.2us) -- park a filler here
                        if urgent:
                            urgent.popleft()()
                        elif lazy:
                            lazy.popleft()()
                    if p >= HPAIRS - 3:
                        # tail: no further slots are guaranteed, flush
                        while urgent:
                            urgent.popleft()()
                        while lazy:
                            lazy.popleft()()

                return proj_units(b)

            carry = []
            for _rep in range(repeat):
                for b in range(B_PC):
                    carry = emit_batch(b, carry)
            for u in carry:
                u()

    return nc


_NC_CACHE = {}


def _get_nc(repeat: int = 1) -> bass.Bass:
    key = f"nc{repeat}"
    if key not in _NC_CACHE:
        _NC_CACHE[key] = build_nc(repeat)
    return _NC_CACHE[key]


def _make_runner(nc):
    """Compile the SPMD kernel ONCE into a reusable callable.

    Mirrors bass2jax.run_bass_via_pjrt's multi-core branch, but without
    output-buffer donation so the compiled function + device-resident
    inputs can be invoked repeatedly (for wall-clock benchmarking and to
    avoid recompiles on every kernel() call).
    """
    import jax
    from jax.experimental.shard_map import shard_map
    from jax.sharding import Mesh, PartitionSpec
    from concourse import bass2jax
    from concourse.bass2jax import _bass_exec_p, partition_id_tensor

    bass2jax.install_neuronx_cc_hook()

    partition_name = (
        nc.partition_id_tensor.name if nc.partition_id_tensor else None
    )
    in_names, out_names, out_avals, zero_outs = [], [], [], []
    for alloc in nc.m.functions[0].allocations:
        if not isinstance(alloc, mybir.MemoryLocationSet):
            continue
        name = alloc.memorylocations[0].name
        if alloc.kind == "ExternalInput":
            if name != partition_name:
                in_names.append(name)
        elif alloc.kind == "ExternalOutput":
            shape = tuple(alloc.tensor_shape)
            dtype = mybir.dt.np(alloc.dtype)
            out_names.append(name)
            out_avals.append(jax.core.ShapedArray(shape, dtype))
            zero_outs.append(np.zeros(shape, dtype))
    n_params = len(in_names)
    all_in_names = list(in_names) + list(out_names)
    if partition_name is not None:
        all_in_names.append(partition_name)

    def _body(*args):
        operands = list(args)
        if partition_name is not None:
            operands.append(partition_id_tensor())
        outs = _bass_exec_p.bind(
            *operands,
            out_avals=tuple(out_avals),
            in_names=tuple(all_in_names),
            out_names=tuple(out_names),
            lowering_input_output_aliases=(),
            sim_require_finite=True,
            sim_require_nnan=True,
            nc=nc,
        )
        return tuple(outs)

    devices = jax.devices()[:N_CORES]
    mesh = Mesh(np.asarray(devices), ("core",))
    n_outs = len(out_avals)
    in_specs = (PartitionSpec("core"),) * (n_params + n_outs)
    out_specs = (PartitionSpec("core"),) * n_outs
    sharded = jax.jit(
        shard_map(_body, mesh=mesh, in_specs=in_specs,
                  out_specs=out_specs, check_rep=False),
        keep_unused=True,
    )

    concat_zeros = [
        np.zeros((N_CORES * z.shape[0], *z.shape[1:]), z.dtype)
        for z in zero_outs
    ]

    state = {"dev_zeros": None}

    def runner(in_maps):
        per_core = [
            [np.asarray(m[name]) for name in in_names] for m in in_maps
        ]
        concat_in = [
            np.concatenate([per_core[c][i] for c in range(N_CORES)], axis=0)
            for i in range(n_params)
        ]
        if state["dev_zeros"] is None:
            state["dev_zeros"] = [jax.device_put(z) for z in concat_zeros]
        out_arrs = sharded(*concat_in, *state["dev_zeros"])
        return [
            {
                name: np.asarray(out_arrs[i]).reshape(
                    N_CORES, *out_avals[i].shape
                )[c]
                for i, name in enumerate(out_names)
            }
            for c in range(N_CORES)
        ]

    def runner_dev(dev_args):
        """dev_args: device-resident concat inputs; returns device outputs."""
        return sharded(*dev_args, *state["dev_zeros"])

    def make_dev_args(in_maps):
        per_core = [
            [np.asarray(m[name]) for name in in_names] for m in in_maps
        ]
        concat_in = [
            np.concatenate([per_core[c][i] for c in range(N_CORES)], axis=0)
            for i in range(n_params)
        ]
        if state["dev_zeros"] is None:
            state["dev_zeros"] = [jax.device_put(z) for z in concat_zeros]
        return [jax.device_put(a) for a in concat_in]

    return runner, runner_dev, make_dev_args


def _get_runner(repeat: int = 1):
    key = f"runner{repeat}"
    if key not in _NC_CACHE:
        _NC_CACHE[key] = _make_runner(_get_nc(repeat))
    return _NC_CACHE[key]


def _make_in_maps(x, pk, pv, w_qkv, w_proj, b_proj):
    x = np.ascontiguousarray(np.asarray(x, dtype=np.float32))
    pk = np.ascontiguousarray(np.asarray(pk, dtype=np.float32))
    pv = np.ascontiguousarray(np.asarray(pv, dtype=np.float32))
    w_qkv = np.ascontiguousarray(np.asarray(w_qkv, dtype=np.float32))
    w_proj = np.ascontiguousarray(np.asarray(w_proj, dtype=np.float32))
    b_proj = np.ascontiguousarray(np.asarray(b_proj, dtype=np.float32))
    in_maps = []
    for c in range(N_CORES):
        sl = slice(c * B_PC, (c + 1) * B_PC)
        in_maps.append({
            "x": x[sl], "pk": pk[sl], "pv": pv[sl],
            "w_qkv": w_qkv, "w_proj": w_proj, "b_proj": b_proj,
        })
    return in_maps


def run(x, pk, pv, w_qkv, w_proj, b_proj, trace=False, **trace_kwargs):
    """Run the SPMD kernel; returns (output [B,N,C], results).

    With trace=True, routes through run_bass_kernel_spmd so the returned
    results object carries .exec_time_ns / .profile_json.
    """
    in_maps = _make_in_maps(x, pk, pv, w_qkv, w_proj, b_proj)
    if trace:
        res = run_bass_kernel_spmd(
            _get_nc(), in_maps, list(range(N_CORES)), trace=True,
            **trace_kwargs,
        )
        results = res.results
        out = np.empty((B, N, C), dtype=np.float32)
        for c in range(N_CORES):
            outT = results[c]["outT"]          # [B_PC, C, N]
            out[c * B_PC:(c + 1) * B_PC] = outT.transpose(0, 2, 1)
        return out, res
    runner, _, _ = _get_runner()
    results = runner(in_maps)
    out = np.empty((B, N, C), dtype=np.float32)
    for c in range(N_CORES):
        outT = results[c]["outT"]              # [B_PC, C, N]
        out[c * B_PC:(c + 1) * B_PC] = outT.transpose(0, 2, 1)
    return out, results


def kernel(x, pk, pv, w_qkv, w_proj, b_proj) -> np.ndarray:
    out, _ = run(x, pk, pv, w_qkv, w_proj, b_proj)
    return out


def benchmark(x, pk, pv, w_qkv, w_proj, b_proj, iters=20, warmup=3, repeat=1):
    """Median wall-clock per executed call with device-resident inputs."""
    import time
    import jax
    _, runner_dev, make_dev_args = _get_runner(repeat)
    in_maps = _make_in_maps(x, pk, pv, w_qkv, w_proj, b_proj)
    dev_args = make_dev_args(in_maps)
    for _ in range(warmup):
        outs = runner_dev(dev_args)
        jax.block_until_ready(outs)
    ts = []
    for _ in range(iters):
        t0 = time.perf_counter()
        outs = runner_dev(dev_args)
        jax.block_until_ready(outs)
        ts.append(time.perf_counter() - t0)
    ts.sort()
    return {
        "median_s": ts[len(ts) // 2],
        "min_s": ts[0],
        "all_s": ts,
    }

